# revision 1
# baseline (speedup 1.0000x reference)
"""Trainium2 Bass kernel for nn_JambaAttentionDecoderLayer (8-core SPMD).

Sharding: tensor-parallel attention (2 q-heads + 1 kv-head per core,
o-proj column-sharded, two AllGathers) + expert parallelism for the MoE
(1 expert per core, dense over tokens, ReduceScatter combine).

Everything on-device is computed in a feature-major ("transposed") layout
[feature, token] so every matmul contraction dim lands on SBUF partitions
without runtime transposes of activations.  Weights are transposed/packed
on the host while sharding.  Big matmuls run in float32r (full PE speed
for free-dim >= 256, ~2^-13 rounding).
"""

import numpy as np

import concourse.bass as bass
import concourse.tile as tile
import concourse.mybir as mybir
from concourse import bacc
from concourse.bass_utils import run_bass_kernel_spmd

# dims (hardcoded per spec)
T = 1024
H = 1024
NH = 16
NKV = 4
HD = 64
I = 2816
E = 8
SW = 512
EPS = 1e-6
SCALE = HD ** -0.5

NCORES = 8
P = 128
KT = H // P          # 8 k-tiles over H
JT = I // P          # 22 k-tiles over I
MT = H // P          # 8 m-tiles over H
NEG = -1.0e30

f32 = mybir.dt.float32
f32r = mybir.dt.float32r

# attention mask offsets: off = q_tile_start - k_tile_start for [128k,512q] tiles
OFFS = [-384, -256, -128, 0, 128, 256, 384, 512]
QT_KIS = {0: list(range(0, 4)), 1: list(range(0, 8))}

AxX = mybir.AxisListType.X
Alu = mybir.AluOpType
Act = mybir.ActivationFunctionType


def _build(profile=False):
    ndev = 1 if profile else NCORES
    nc = bacc.Bacc("TRN2", target_bir_lowering=False, debug=False,
                   num_devices=ndev)

    # ---- kernel I/O ----
    hT_d = nc.dram_tensor("hT", [H, T], f32, kind="ExternalInput")
    qkvwT_d = nc.dram_tensor("qkvwT", [H, 256], f32r, kind="ExternalInput")
    owT_d = nc.dram_tensor("owT", [H, P], f32, kind="ExternalInput")
    ln1w_d = nc.dram_tensor("ln1w", [P, KT], f32, kind="ExternalInput")
    ln2w_d = nc.dram_tensor("ln2w", [P, KT], f32, kind="ExternalInput")
    rwT_d = nc.dram_tensor("rwT", [P, KT, E], f32r, kind="ExternalInput")
    wsp_d = nc.dram_tensor("wsp", [2 * JT, P, KT * P], f32r, kind="ExternalInput")
    w2p_d = nc.dram_tensor("w2p", [MT, P, JT * P], f32r, kind="ExternalInput")
    amask_d = nc.dram_tensor("amask", [len(OFFS), P, 512], mybir.dt.bfloat16,
                             kind="ExternalInput")
    onehot_d = nc.dram_tensor("onehot", [E, 1], f32r, kind="ExternalInput")
    ones128_d = nc.dram_tensor("ones128", [P, 1], f32r, kind="ExternalInput")
    ones1r_d = nc.dram_tensor("ones1r", [1, P], f32r, kind="ExternalInput")

    moe_sl_d = nc.dram_tensor("moe_slice", [P, T], f32, kind="ExternalOutput")
    residT_d = nc.dram_tensor("residT", [H, T], f32, kind="ExternalOutput")

    rg = [list(range(NCORES))]

    import contextlib
    lp = getattr(nc, "allow_low_precision", None)
    lp_cm = lp(reason="float32r matmul operands; rounding ~2^-13 acceptable") \
        if lp else contextlib.nullcontext()
    with lp_cm, tile.TileContext(nc) as tc:
        with tc.tile_pool(name="const", bufs=1) as cpool, \
             tc.tile_pool(name="persist", bufs=1) as pers, \
             tc.tile_pool(name="dram", bufs=1, space="DRAM") as dram:

            # ---- constants ----
            ones128 = cpool.tile([P, 1], f32r)
            nc.sync.dma_start(ones128[:], ones128_d[:])
            ones1r = cpool.tile([1, P], f32r)
            nc.sync.dma_start(ones1r[:], ones1r_d[:])
            onehot = cpool.tile([E, 1], f32r)
            nc.sync.dma_start(onehot[:], onehot_d[:])
            ln1w = cpool.tile([P, KT], f32)
            nc.sync.dma_start(ln1w[:], ln1w_d[:])
            ln2w = cpool.tile([P, KT], f32)
            nc.sync.dma_start(ln2w[:], ln2w_d[:])
            ident = cpool.tile([P, P], f32)
            from concourse.masks import make_identity
            make_identity(nc, ident[:])

            # dram bounce buffers for collectives
            ag1_in = dram.tile([P, T], f32)
            ag1_out = dram.tile([H, T], f32, addr_space="Shared")
            ag2_in = dram.tile([P, T], f32)
            ag2_out = dram.tile([H, T], f32, addr_space="Shared")
            rs_in = dram.tile([H, T], f32)
            rs_out = dram.tile([P, T], f32)

            # =========== RMSNorm helper (feature-major) ===========
            def rmsnorm(src_tile, lnw_tile, dst_tile):
                with tc.tile_pool(name="rn", bufs=1) as tmp, \
                     tc.tile_pool(name="rnps", bufs=1, space="PSUM") as psum:
                    vs = [None, None]
                    for ni in range(2):
                        pv = psum.tile([1, 512], f32, tag="pvar")
                        for k in range(KT):
                            sq = tmp.tile([P, 512], f32r, tag="sq", bufs=2)
                            nc.scalar.activation(
                                sq[:], src_tile[:, k, ni * 512:(ni + 1) * 512],
                                Act.Square)
                            nc.tensor.matmul(pv[:], ones128[:], sq[:],
                                             start=(k == 0), stop=(k == KT - 1))
                        v = tmp.tile([1, 512], f32, tag="vv")
                        nc.vector.tensor_scalar(v[:], pv[:], 1.0 / H, EPS,
                                                Alu.mult, Alu.add)
                        sd = tmp.tile([1, 512], f32, tag="sd")
                        nc.scalar.activation(sd[:], v[:], Act.Sqrt)
                        s = tmp.tile([1, 512], f32r, tag="ss")
                        nc.vector.reciprocal(s[:], sd[:])
                        pb = psum.tile([P, 512], f32, tag="pbc", bufs=2)
                        nc.tensor.matmul(pb[:], ones1r[:], s[:],
                                         start=True, stop=True)
                        vs[ni] = pb
                    for ni in range(2):
                        for k in range(KT):
                            nc.vector.scalar_tensor_tensor(
                                dst_tile[:, k, ni * 512:(ni + 1) * 512],
                                src_tile[:, k, ni * 512:(ni + 1) * 512],
                                lnw_tile[:, k:k + 1],
                                vs[ni][:],
                                Alu.mult, Alu.mult)

            # =========== phase 1+2: attention (needs hT) ===========
            with tc.tile_pool(name="residp", bufs=1) as residp:
                with tc.tile_pool(name="hp", bufs=1) as hp:
                    hT = hp.tile([P, KT, T], f32)
                    nc.sync.dma_start(hT[:], hT_d.rearrange("(k p) t -> p k t", p=P))

                    # ---- ln1 + qkv + attention ----
                    with tc.tile_pool(name="p1", bufs=2) as p1:
                        qkvT = p1.tile([P, 2, T], f32r, bufs=1)
                        with tc.tile_pool(name="p1a", bufs=1) as p1a:
                            hnT = p1a.tile([P, KT, T], f32r)
                            rmsnorm(hT, ln1w, hnT)

                            with tc.tile_pool(name="ps1", bufs=1, space="PSUM") as ps1:
                                qkvw = p1a.tile([P, KT, 256], f32r)
                                nc.sync.dma_start(
                                    qkvw[:], qkvwT_d.rearrange("(k p) m -> p k m", p=P))
                                for mi in range(2):
                                    for ni in range(2):
                                        pq = ps1.tile([P, 512], f32, tag="pqkv", bufs=2)
                                        for k in range(KT):
                                            nc.tensor.matmul(
                                                pq[:], qkvw[:, k, mi * P:(mi + 1) * P],
                                                hnT[:, k, ni * 512:(ni + 1) * 512],
                                                start=(k == 0), stop=(k == KT - 1))
                                        nc.vector.tensor_copy(
                                            qkvT[:, mi, ni * 512:(ni + 1) * 512], pq[:])

                        # v to token-major [128tok, 8tiles, 64]
                        v_sb = p1.tile([P, KT, HD], f32r, bufs=1)
                        with tc.tile_pool(name="ps1v", bufs=1, space="PSUM") as ps1v:
                            for ti in range(KT):
                                pvt = ps1v.tile([P, HD], f32, tag="pvt", bufs=2)
                                nc.tensor.transpose(
                                    pvt[:],
                                    qkvT[HD:P, 1, ti * P:(ti + 1) * P].bitcast(f32),
                                    ident[HD:P, HD:P])
                                nc.vector.tensor_copy(v_sb[:, ti, :], pvt[:])

                        attn_sb = pers.tile([HD, 2, T], f32)
                        am = p1.tile([P, len(OFFS), 512], mybir.dt.bfloat16, bufs=1)
                        nc.sync.dma_start(am[:], amask_d.rearrange("o p f -> p o f"))

                        # re-base head-1 q to partitions 0..63 (SBUF->SBUF DMA)
                        q1_sb = p1.tile([HD, T], f32r, bufs=1)
                        nc.sync.dma_start(q1_sb[:], qkvT[HD:P, 0, :])

                        with tc.tile_pool(name="ps1b", bufs=1, space="PSUM") as ps1b:
                            for h in range(2):
                                qT = qkvT[0:HD, 0, :] if h == 0 else q1_sb[:]
                                kTT = qkvT[0:HD, 1, :]
                                for qt in range(2):
                                    kis = QT_KIS[qt]
                                    ppv = ps1b.tile([HD, 512], f32, tag="ppv")
                                    pcs = ps1b.tile([1, 512], f32, tag="pcs")
                                    for idx, ki in enumerate(kis):
                                        pscore = ps1b.tile([P, 512], f32,
                                                           tag="pscore", bufs=2)
                                        nc.tensor.matmul(
                                            pscore[:], kTT[:, ki * P:(ki + 1) * P],
                                            qT[:, qt * 512:(qt + 1) * 512],
                                            start=True, stop=True)
                                        off_i = OFFS.index(qt * 512 - ki * P)
                                        sm = p1.tile([P, 512], f32, tag="sm")
                                        nc.vector.scalar_tensor_tensor(
                                            sm[:], pscore[:], SCALE,
                                            am[:, off_i, :], Alu.mult, Alu.add)
                                        pexp = p1.tile([P, 512], f32r, tag="pexp")
                                        nc.scalar.activation(pexp[:], sm[:], Act.Exp)
                                        nc.tensor.matmul(
                                            pcs[:], ones128[:], pexp[:],
                                            start=(idx == 0),
                                            stop=(idx == len(kis) - 1))
                                        nc.tensor.matmul(
                                            ppv[:], v_sb[:, ki, :], pexp[:],
                                            start=(idx == 0),
                                            stop=(idx == len(kis) - 1))
                                    inv = p1.tile([1, 512], f32r, tag="inv")
                                    nc.vector.reciprocal(inv[:], pcs[:])
                                    pbc = ps1b.tile([P, 512], f32, tag="pbc2")
                                    nc.tensor.matmul(pbc[:], ones1r[:], inv[:],
                                                     start=True, stop=True)
                                    binv = p1.tile([HD, 512], f32, tag="binv")
                                    nc.vector.tensor_copy(binv[:], pbc[:HD, :])
                                    nc.vector.tensor_tensor(
                                        attn_sb[:, h, qt * 512:(qt + 1) * 512],
                                        ppv[:], binv[:], Alu.mult)

                        nc.sync.dma_start(
                            ag1_in[:].rearrange("(h d) t -> d h t", h=2), attn_sb[:])
                        if not profile:
                            nc.gpsimd.collective_compute(
                                "AllGather", Alu.bypass, replica_groups=rg,
                                ins=[ag1_in[:]], outs=[ag1_out[:]])

                    # ---- o-proj (fp32) + AG2 + residual ----
                    with tc.tile_pool(name="p2", bufs=2) as p2, \
                         tc.tile_pool(name="ps2", bufs=2, space="PSUM") as ps2:
                        ow = p2.tile([P, KT, P], f32, bufs=1)
                        nc.sync.dma_start(
                            ow[:], owT_d.rearrange("(k p) m -> p k m", p=P))
                        af = p2.tile([P, KT, T], f32, bufs=1)
                        nc.sync.dma_start(
                            af[:], ag1_out.rearrange("(k p) t -> p k t", p=P))
                        ao_sl = p2.tile([P, T], f32, bufs=1)
                        for ni in range(2):
                            po = ps2.tile([P, 512], f32, tag="po")
                            for k in range(KT):
                                nc.tensor.matmul(
                                    po[:], ow[:, k, :],
                                    af[:, k, ni * 512:(ni + 1) * 512],
                                    start=(k == 0), stop=(k == KT - 1))
                            nc.vector.tensor_copy(
                                ao_sl[:, ni * 512:(ni + 1) * 512], po[:])
                        nc.sync.dma_start(ag2_in[:], ao_sl[:])
                        if not profile:
                            nc.gpsimd.collective_compute(
                                "AllGather", Alu.bypass, replica_groups=rg,
                                ins=[ag2_in[:]], outs=[ag2_out[:]])

                        residT = residp.tile([P, KT, T], f32)
                        nc.sync.dma_start(
                            residT[:], ag2_out.rearrange("(k p) t -> p k t", p=P))
                        for k in range(KT):
                            nc.vector.tensor_add(residT[:, k, :], hT[:, k, :],
                                                 residT[:, k, :])
                        nc.sync.dma_start(
                            residT_d.rearrange("(k p) t -> p k t", p=P), residT[:])
                # hT pool closed here

                # =========== phase 3: ln2 + router + top2 weights ===========
                h2T = pers.tile([P, KT, T], f32r)
                wb = pers.tile([P, T], f32)
                rmsnorm(residT, ln2w, h2T)

                with tc.tile_pool(name="p3", bufs=2) as p3, \
                     tc.tile_pool(name="ps3", bufs=1, space="PSUM") as ps3:
                    rw = p3.tile([P, KT, E], f32r, bufs=1)
                    nc.sync.dma_start(rw[:], rwT_d[:])
                    logT = p3.tile([E, T], f32, bufs=1)
                    for ni in range(2):
                        pr = ps3.tile([E, 512], f32, tag="pr", bufs=2)
                        for k in range(KT):
                            nc.tensor.matmul(pr[:], rw[:, k, :],
                                             h2T[:, k, ni * 512:(ni + 1) * 512],
                                             start=(k == 0), stop=(k == KT - 1))
                        nc.vector.tensor_copy(logT[:, ni * 512:(ni + 1) * 512],
                                              pr[:])

                    wT = p3.tile([E, T], f32r, bufs=1)
                    for ti in range(KT):
                        ptr = ps3.tile([P, E], f32, tag="ptr", bufs=2)
                        nc.tensor.transpose(ptr[:], logT[:, ti * P:(ti + 1) * P],
                                            ident[:E, :E])
                        lg = p3.tile([P, E], f32, tag="lg")
                        nc.vector.tensor_copy(lg[:], ptr[:])
                        m1 = p3.tile([P, 1], f32, tag="m1")
                        nc.vector.reduce_max(m1[:], lg[:], axis=AxX)
                        nm1 = p3.tile([P, 1], f32, tag="nm1")
                        nc.vector.tensor_scalar_mul(nm1[:], m1[:], -1.0)
                        ex = p3.tile([P, E], f32, tag="ex")
                        nc.scalar.activation(ex[:], lg[:], Act.Exp, bias=nm1[:])
                        den = p3.tile([P, 1], f32, tag="den")
                        nc.vector.reduce_sum(den[:], ex[:], axis=AxX)
                        inv2 = p3.tile([P, 1], f32, tag="inv2")
                        nc.vector.reciprocal(inv2[:], den[:])
                        eq = p3.tile([P, E], f32, tag="eq")
                        nc.vector.tensor_scalar(eq[:], lg[:], m1[:], None,
                                                Alu.is_equal)
                        msk = p3.tile([P, E], f32, tag="msk")
                        nc.vector.scalar_tensor_tensor(msk[:], eq[:], NEG, lg[:],
                                                       Alu.mult, Alu.add)
                        m2 = p3.tile([P, 1], f32, tag="m2")
                        nc.vector.reduce_max(m2[:], msk[:], axis=AxX)
                        sel = p3.tile([P, E], f32, tag="sel")
                        nc.vector.tensor_scalar(sel[:], lg[:], m2[:], None,
                                                Alu.is_ge)
                        wtm = p3.tile([P, E], f32, tag="wtm")
                        nc.vector.tensor_scalar_mul(wtm[:], ex[:], inv2[:])
                        nc.vector.tensor_tensor(wtm[:], wtm[:], sel[:], Alu.mult)
                        pwt = ps3.tile([E, P], f32, tag="pwt", bufs=2)
                        nc.tensor.transpose(pwt[:], wtm[:], ident[:])
                        nc.vector.tensor_copy(wT[:, ti * P:(ti + 1) * P], pwt[:])

                    wrow = p3.tile([1, T], f32r, bufs=1)
                    for ni in range(2):
                        pwr = ps3.tile([1, 512], f32, tag="pwr")
                        nc.tensor.matmul(pwr[:], onehot[:],
                                         wT[:, ni * 512:(ni + 1) * 512],
                                         start=True, stop=True)
                        nc.vector.tensor_copy(wrow[:, ni * 512:(ni + 1) * 512],
                                              pwr[:])
                    for ni in range(2):
                        pwb = ps3.tile([P, 512], f32, tag="pwb")
                        nc.tensor.matmul(pwb[:], ones1r[:],
                                         wrow[:, ni * 512:(ni + 1) * 512],
                                         start=True, stop=True)
                        nc.vector.tensor_copy(wb[:, ni * 512:(ni + 1) * 512],
                                              pwb[:])
            # residT pool closed here

            # =========== phase 4: expert FFN (dense over T) ===========
            with tc.tile_pool(name="wpool", bufs=2) as wpool, \
                 tc.tile_pool(name="apool", bufs=1) as apool, \
                 tc.tile_pool(name="spool", bufs=2) as spool, \
                 tc.tile_pool(name="ps4", bufs=1, space="PSUM") as ps4:
                act = apool.tile([P, JT, T], f32r)
                for j in range(JT):
                    wg = wpool.tile([P, KT * P], f32r, tag="wg", bufs=2)
                    nc.sync.dma_start(wg[:], wsp_d[j])
                    wu = wpool.tile([P, KT * P], f32r, tag="wu", bufs=2)
                    nc.sync.dma_start(wu[:], wsp_d[JT + j])
                    for ni in range(2):
                        pg = ps4.tile([P, 512], f32, tag=f"pg{ni}")
                        pu = ps4.tile([P, 512], f32, tag=f"pu{ni}")
                        for k in range(KT):
                            nc.tensor.matmul(pg[:], wg[:, k * P:(k + 1) * P],
                                             h2T[:, k, ni * 512:(ni + 1) * 512],
                                             start=(k == 0), stop=(k == KT - 1))
                        for k in range(KT):
                            nc.tensor.matmul(pu[:], wu[:, k * P:(k + 1) * P],
                                             h2T[:, k, ni * 512:(ni + 1) * 512],
                                             start=(k == 0), stop=(k == KT - 1))
                        sil = spool.tile([P, 512], f32, tag="sil")
                        nc.scalar.activation(sil[:], pg[:], Act.Silu)
                        nc.vector.tensor_tensor(act[:, j, ni * 512:(ni + 1) * 512],
                                                sil[:], pu[:], Alu.mult)

                for m in range(MT):
                    w2 = wpool.tile([P, JT * P], f32r, tag="w2", bufs=2)
                    nc.sync.dma_start(w2[:], w2p_d[m])
                    for ni in range(2):
                        pd = ps4.tile([P, 512], f32, tag="pd", bufs=2)
                        for j in range(JT):
                            nc.tensor.matmul(pd[:], w2[:, j * P:(j + 1) * P],
                                             act[:, j, ni * 512:(ni + 1) * 512],
                                             start=(j == 0), stop=(j == JT - 1))
                        eo = spool.tile([P, 512], f32, tag="eo")
                        nc.vector.tensor_tensor(eo[:], pd[:],
                                                wb[:, ni * 512:(ni + 1) * 512],
                                                Alu.mult)
                        nc.sync.dma_start(rs_in[m * P:(m + 1) * P,
                                                ni * 512:(ni + 1) * 512], eo[:])

                if not profile:
                    nc.gpsimd.collective_compute(
                        "ReduceScatter", Alu.add, replica_groups=rg,
                        ins=[rs_in[:]], outs=[rs_out[:]])
                out_sb = spool.tile([P, T], f32, tag="osb")
                nc.sync.dma_start(out_sb[:], rs_out[:])
                nc.sync.dma_start(moe_sl_d[:], out_sb[:])

    nc.compile()
    return nc


_NC = None


def _get_nc():
    global _NC
    if _NC is None:
        _NC = _build()
    return _NC


def _pack_inputs(hidden_states, ln1_w, qkv_w, o_w, ln2_w, router_w, ws, w2s):
    hidden_states = np.asarray(hidden_states, np.float32)
    qkv_w = np.asarray(qkv_w, np.float32)
    o_w = np.asarray(o_w, np.float32)
    router_w = np.asarray(router_w, np.float32)
    ws = np.asarray(ws, np.float32)
    w2s = np.asarray(w2s, np.float32)
    ln1_w = np.asarray(ln1_w, np.float32)
    ln2_w = np.asarray(ln2_w, np.float32)

    hT = np.ascontiguousarray(hidden_states.T)
    ln1p = np.ascontiguousarray(ln1_w.reshape(KT, P).T)
    ln2p = np.ascontiguousarray(ln2_w.reshape(KT, P).T)
    rwT = np.ascontiguousarray(
        router_w.T.reshape(KT, P, E).transpose(1, 0, 2))

    amask = np.empty((len(OFFS), P, 512), np.float32)
    pp = np.arange(P)[:, None]
    ff = np.arange(512)[None, :]
    for i, off in enumerate(OFFS):
        d = off + ff - pp
        amask[i] = np.where((d >= 0) & (d < SW), 0.0, NEG)
    import ml_dtypes
    amask = amask.astype(ml_dtypes.bfloat16)

    ones128 = np.ones((P, 1), np.float32)
    ones1r = np.ones((1, P), np.float32)

    in_maps = []
    for c in range(NCORES):
        qrows = qkv_w[2 * c * HD:(2 * c + 2) * HD]
        krows = qkv_w[NH * HD + (c // 2) * HD: NH * HD + (c // 2 + 1) * HD]
        vrows = qkv_w[(NH + NKV) * HD + (c // 2) * HD:
                      (NH + NKV) * HD + (c // 2 + 1) * HD]
        qkv_sh = np.concatenate([qrows, krows, vrows], axis=0)   # [256, H]
        qkvwT = np.ascontiguousarray(qkv_sh.T)                   # [H, 256]
        owT = np.ascontiguousarray(o_w[c * P:(c + 1) * P, :].T)  # [NH*HD, 128]

        wsT = ws[c].T                                            # [H, 2I]
        wsp = np.ascontiguousarray(
            wsT.reshape(KT, P, 2 * JT, P).transpose(2, 1, 0, 3)
               .reshape(2 * JT, P, KT * P))
        w2T = w2s[c].T                                           # [I, H]
        w2p = np.ascontiguousarray(
            w2T.reshape(JT, P, MT, P).transpose(2, 1, 0, 3)
               .reshape(MT, P, JT * P))

        onehot = np.zeros((E, 1), np.float32)
        onehot[c, 0] = 1.0

        in_maps.append({
            "hT": hT, "qkvwT": qkvwT, "owT": owT,
            "ln1w": ln1p, "ln2w": ln2p, "rwT": rwT,
            "wsp": wsp, "w2p": w2p, "amask": amask,
            "onehot": onehot, "ones128": ones128, "ones1r": ones1r,
        })
    return in_maps


def kernel(hidden_states, positions, ln1_w, qkv_w, o_w, ln2_w, router_w, ws, w2s):
    nc = _get_nc()
    in_maps = _pack_inputs(hidden_states, ln1_w, qkv_w, o_w, ln2_w,
                           router_w, ws, w2s)
    res = run_bass_kernel_spmd(nc, in_maps, list(range(NCORES)))
    moe_T = np.concatenate([res.results[c]["moe_slice"] for c in range(NCORES)],
                           axis=0)                               # [H, T]
    moe_out = np.ascontiguousarray(moe_T.T)
    residual = np.ascontiguousarray(res.results[0]["residT"].T)
    return moe_out, residual



# revision 45
# speedup vs baseline: 1.0739x; 1.0739x over previous
"""Trainium2 Bass kernel for nn_JambaAttentionDecoderLayer (8-core SPMD).

v2: routed MoE + restructured collectives.

Sharding: tensor-parallel attention (2 q-heads + 1 kv-head per core) with an
AllToAll feature->token exchange, token-sliced o-proj/rmsnorm/router
(128 tokens per core), and expert parallelism for the MoE with true top-2
token routing: per-expert token index lists are built on device with
matmul-based stream compaction, tokens gathered with the DGE dma_gather
(transpose mode, bf16), expert FFN runs on a fixed 384-token capacity
(vs 1024 dense), and results are scattered back with a 0/1 scatter-matmul
followed by a bf16 ReduceScatter.

Precision: the attention -> residual -> rmsnorm -> router-logits path stays
in f32/f32r so the top-2 expert *selection* matches the f32 reference
(min |logit2-logit3| gap for this model is ~5e-4; bf16 noise would flip
experts). Everything downstream of selection (expert inputs, weights,
combine) runs in bf16: errors there scale with the element magnitude and
stay well inside the 2e-2 gate.
"""

import os as _os
import numpy as np

import concourse.bass as bass
import concourse.tile as tile
import concourse.mybir as mybir
from concourse import bacc
from concourse.bass_utils import run_bass_kernel_spmd

# dims (hardcoded per spec)
T = 1024
H = 1024
NH = 16
NKV = 4
HD = 64
I = 2816
E = 8
SW = 512
EPS = 1e-6
SCALE = HD ** -0.5

NCORES = 8
P = 128
KT = H // P          # 8 k-tiles over H
JT = I // P          # 22 j-tiles over I
C = 384              # MoE token capacity per expert (max count here is 275)
RC = C // P          # 3 slot chunks
NEG = -1.0e30

f32 = mybir.dt.float32
f32r = mybir.dt.float32r
bf16 = mybir.dt.bfloat16
i16 = mybir.dt.int16

# attention mask offsets: off = q_tile_start - k_tile_start for [128k,512q] tiles
OFFS = [-384, -256, -128, 0, 128, 256, 384, 512]
QT_KIS = {0: list(range(0, 4)), 1: list(range(0, 8))}

AxX = mybir.AxisListType.X
Alu = mybir.AluOpType
Act = mybir.ActivationFunctionType


def _build(profile=False):
    ndev = 1 if profile else NCORES
    nc = bacc.Bacc("TRN2", target_bir_lowering=False, debug=False,
                   num_devices=ndev)

    # ---- kernel I/O ----
    hT_d = nc.dram_tensor("hT", [H, T], f32, kind="ExternalInput")
    htok_d = nc.dram_tensor("htok", [P, H], f32, kind="ExternalInput")
    qkvwT_d = nc.dram_tensor("qkvwT", [H, 256], f32r, kind="ExternalInput")
    owTf_d = nc.dram_tensor("owTf", [P, KT, H], f32r, kind="ExternalInput")
    ln1w_d = nc.dram_tensor("ln1w", [P, KT], f32, kind="ExternalInput")
    ln2row_d = nc.dram_tensor("ln2row", [1, H], f32r, kind="ExternalInput")
    rwT_d = nc.dram_tensor("rwT", [P, KT, E], f32, kind="ExternalInput")
    wspb_d = nc.dram_tensor("wspb", [2 * JT, P, KT * P], bf16,
                            kind="ExternalInput")
    w2tp_d = nc.dram_tensor("w2tp", [P, JT, H], bf16, kind="ExternalInput")
    amask_d = nc.dram_tensor("amask", [len(OFFS), P, 512], bf16,
                             kind="ExternalInput")
    iota_row_d = nc.dram_tensor("iota_row", [1, T], f32r, kind="ExternalInput")
    ones128_d = nc.dram_tensor("ones128", [P, 1], f32r, kind="ExternalInput")
    ones1r_d = nc.dram_tensor("ones1r", [1, P], f32r, kind="ExternalInput")
    colc_d = nc.dram_tensor("colc", [P, 1], f32, kind="ExternalInput")
    tokidx_d = nc.dram_tensor("tokidx", [P, KT], f32r, kind="ExternalInput")
    l128_d = nc.dram_tensor("l128", [P, P], f32r, kind="ExternalInput")
    u8_d = nc.dram_tensor("u8", [E, E], f32r, kind="ExternalInput")
    ones2_d = nc.dram_tensor("ones2", [P, 2], f32r, kind="ExternalInput")

    resid_sl_d = nc.dram_tensor("resid_sl", [P, H], f32, kind="ExternalOutput")
    moe_sl_d = nc.dram_tensor("moe_sl", [P, H], bf16, kind="ExternalOutput")
    cnt_d = nc.dram_tensor("cnt", [1, 1], f32, kind="ExternalOutput")
    dbg_wtm_d = nc.dram_tensor("dbg_wtm", [P, 4], f32, kind="ExternalOutput")
    dbg_pos_d = nc.dram_tensor("dbg_pos", [1, C], f32, kind="ExternalOutput")
    dbg_gat_d = nc.dram_tensor("dbg_gat", [P, RC], f32, kind="ExternalOutput")
    dbg_h2c_d = nc.dram_tensor("dbg_h2c", [P, KT, 8], f32, kind="ExternalOutput")

    rg = [list(range(NCORES))]

    import contextlib
    lp = getattr(nc, "allow_low_precision", None)
    lp_cm = lp(reason="f32r/bf16 matmul operands; within rel-err budget") \
        if lp else contextlib.nullcontext()
    with lp_cm, tile.TileContext(nc) as tc:
        with tc.tile_pool(name="const", bufs=1) as cpool, \
             tc.tile_pool(name="persist", bufs=1) as pers, \
             tc.tile_pool(name="dram", bufs=1, space="DRAM") as dram:

            # ---- constants ----
            ones128 = cpool.tile([P, 1], f32r)
            nc.sync.dma_start(ones128[:], ones128_d[:])
            ones1r = cpool.tile([1, P], f32r)
            nc.sync.dma_start(ones1r[:], ones1r_d[:])
            ln1w = cpool.tile([P, KT], f32)
            nc.sync.dma_start(ln1w[:], ln1w_d[:])
            colc = cpool.tile([P, 1], f32)
            nc.sync.dma_start(colc[:], colc_d[:])
            tokidx = cpool.tile([P, KT], f32r)
            nc.sync.dma_start(tokidx[:], tokidx_d[:])
            l128 = cpool.tile([P, P], f32r)
            nc.sync.dma_start(l128[:], l128_d[:])
            u8 = cpool.tile([E, E], f32r)
            nc.sync.dma_start(u8[:], u8_d[:])
            ones2 = cpool.tile([P, 2], f32r)
            nc.sync.dma_start(ones2[:], ones2_d[:])
            ident = cpool.tile([P, P], f32)
            from concourse.masks import make_identity
            make_identity(nc, ident[:])
            iota_row = cpool.tile([1, T], f32r)
            nc.sync.dma_start(iota_row[:], iota_row_d[:])

            htok = pers.tile([P, H], f32)
            nc.sync.dma_start(htok[:], htok_d[:])
            rwT = pers.tile([P, KT, E], f32)
            nc.sync.dma_start(rwT[:], rwT_d[:])
            ln2row = pers.tile([1, H], f32r)
            nc.sync.dma_start(ln2row[:], ln2row_d[:])

            # broadcast rows: iota [128, T] and ln2w [128, H]
            iotaT = pers.tile([P, T], f32)
            ln2bc = pers.tile([P, H], f32)
            with tc.tile_pool(name="bc", bufs=1, space="PSUM") as bcps:
                for ni in range(2):
                    pbi = bcps.tile([P, 512], f32, tag="pbi", bufs=2)
                    nc.tensor.matmul(pbi[:], ones1r[:],
                                     iota_row[:, ni * 512:(ni + 1) * 512],
                                     start=True, stop=True)
                    nc.vector.tensor_copy(iotaT[:, ni * 512:(ni + 1) * 512],
                                          pbi[:])
                    pbl = bcps.tile([P, 512], f32, tag="pbl", bufs=2)
                    nc.tensor.matmul(pbl[:], ones1r[:],
                                     ln2row[:, ni * 512:(ni + 1) * 512],
                                     start=True, stop=True)
                    nc.vector.tensor_copy(ln2bc[:, ni * 512:(ni + 1) * 512],
                                          pbl[:])

            # dram bounce buffers for collectives
            a2a_in = dram.tile([KT, P, P], f32)
            a2a_out = dram.tile([KT, P, P], f32)
            ag2a_in = dram.tile([P, H], bf16)
            ag2a_out = dram.tile([T, H], bf16, addr_space="Shared")
            ag2b_in = dram.tile([P, 32], f32)
            ag2b_out = dram.tile([T, 32], f32, addr_space="Shared")
            idx_d = dram.tile([1, C], i16)
            rs_in = dram.tile([T, H], bf16)
            rs_out = dram.tile([P, H], bf16)

            # =========== feature-major RMSNorm (ln1), as baseline ===========
            def rmsnorm_fm(src_tile, lnw_tile, dst_tile):
                with tc.tile_pool(name="rn", bufs=1) as tmp, \
                     tc.tile_pool(name="rnps", bufs=1, space="PSUM") as psum:
                    vs = [None, None]
                    for ni in range(2):
                        pv = psum.tile([1, 512], f32, tag="pvar")
                        for k in range(KT):
                            sq = tmp.tile([P, 512], f32r, tag="sq", bufs=2)
                            nc.scalar.activation(
                                sq[:], src_tile[:, k, ni * 512:(ni + 1) * 512],
                                Act.Square)
                            nc.tensor.matmul(pv[:], ones128[:], sq[:],
                                             start=(k == 0), stop=(k == KT - 1))
                        v = tmp.tile([1, 512], f32, tag="vv")
                        nc.vector.tensor_scalar(v[:], pv[:], 1.0 / H, EPS,
                                                Alu.mult, Alu.add)
                        sd = tmp.tile([1, 512], f32, tag="sd")
                        nc.scalar.activation(sd[:], v[:], Act.Sqrt)
                        s = tmp.tile([1, 512], f32r, tag="ss")
                        nc.vector.reciprocal(s[:], sd[:])
                        pb = psum.tile([P, 512], f32, tag="pbc", bufs=2)
                        nc.tensor.matmul(pb[:], ones1r[:], s[:],
                                         start=True, stop=True)
                        vs[ni] = pb
                    for ni in range(2):
                        for k in range(KT):
                            nc.vector.scalar_tensor_tensor(
                                dst_tile[:, k, ni * 512:(ni + 1) * 512],
                                src_tile[:, k, ni * 512:(ni + 1) * 512],
                                lnw_tile[:, k:k + 1],
                                vs[ni][:],
                                Alu.mult, Alu.mult)

            # =========== phase 1: ln1 + qkv + attention (f32r, as baseline) ==
            with tc.tile_pool(name="hp", bufs=1) as hp:
                hT = hp.tile([P, KT, T], f32)
                nc.sync.dma_start(hT[:], hT_d.rearrange("(k p) t -> p k t", p=P))

                with tc.tile_pool(name="p1", bufs=2) as p1:
                    qkvT = p1.tile([P, 2, T], f32r, bufs=1)
                    with tc.tile_pool(name="p1a", bufs=1) as p1a:
                        hnT = p1a.tile([P, KT, T], f32r)
                        rmsnorm_fm(hT, ln1w, hnT)

                        with tc.tile_pool(name="ps1", bufs=1, space="PSUM") as ps1:
                            qkvw = p1a.tile([P, KT, 256], f32r)
                            nc.sync.dma_start(
                                qkvw[:], qkvwT_d.rearrange("(k p) m -> p k m", p=P))
                            for mi in range(2):
                                for ni in range(2):
                                    pq = ps1.tile([P, 512], f32, tag="pqkv", bufs=2)
                                    for k in range(KT):
                                        nc.tensor.matmul(
                                            pq[:], qkvw[:, k, mi * P:(mi + 1) * P],
                                            hnT[:, k, ni * 512:(ni + 1) * 512],
                                            start=(k == 0), stop=(k == KT - 1))
                                    nc.vector.tensor_copy(
                                        qkvT[:, mi, ni * 512:(ni + 1) * 512], pq[:])

                    # v to token-major [128tok, 8tiles, 64]
                    v_sb = p1.tile([P, KT, HD], f32r, bufs=1)
                    with tc.tile_pool(name="ps1v", bufs=1, space="PSUM") as ps1v:
                        for ti in range(KT):
                            pvt = ps1v.tile([P, HD], f32, tag="pvt", bufs=2)
                            nc.tensor.transpose(
                                pvt[:],
                                qkvT[HD:P, 1, ti * P:(ti + 1) * P].bitcast(f32),
                                ident[HD:P, HD:P])
                            nc.vector.tensor_copy(v_sb[:, ti, :], pvt[:])

                    attn_sb = p1.tile([HD, 2, T], f32r, bufs=1)
                    am = p1.tile([P, len(OFFS), 512], bf16, bufs=1)
                    nc.sync.dma_start(am[:], amask_d.rearrange("o p f -> p o f"))

                    # re-base head-1 q to partitions 0..63 (SBUF->SBUF DMA)
                    q1_sb = p1.tile([HD, T], f32r, bufs=1)
                    nc.sync.dma_start(q1_sb[:], qkvT[HD:P, 0, :])

                    with tc.tile_pool(name="ps1b", bufs=1, space="PSUM") as ps1b:
                        for h in range(2):
                            qT = qkvT[0:HD, 0, :] if h == 0 else q1_sb[:]
                            kTT = qkvT[0:HD, 1, :]
                            for qt in range(2):
                                kis = QT_KIS[qt]
                                ppv = ps1b.tile([HD, 512], f32, tag="ppv")
                                pcs = ps1b.tile([1, 512], f32, tag="pcs")
                                for idx, ki in enumerate(kis):
                                    pscore = ps1b.tile([P, 512], f32,
                                                       tag="pscore", bufs=2)
                                    nc.tensor.matmul(
                                        pscore[:], kTT[:, ki * P:(ki + 1) * P],
                                        qT[:, qt * 512:(qt + 1) * 512],
                                        start=True, stop=True)
                                    off_i = OFFS.index(qt * 512 - ki * P)
                                    sm = p1.tile([P, 512], f32, tag="sm")
                                    nc.vector.scalar_tensor_tensor(
                                        sm[:], pscore[:], SCALE,
                                        am[:, off_i, :], Alu.mult, Alu.add)
                                    pexp = p1.tile([P, 512], f32r, tag="pexp")
                                    nc.scalar.activation(pexp[:], sm[:], Act.Exp)
                                    nc.tensor.matmul(
                                        pcs[:], ones128[:], pexp[:],
                                        start=(idx == 0),
                                        stop=(idx == len(kis) - 1))
                                    nc.tensor.matmul(
                                        ppv[:], v_sb[:, ki, :], pexp[:],
                                        start=(idx == 0),
                                        stop=(idx == len(kis) - 1))
                                inv = p1.tile([1, 512], f32r, tag="inv")
                                nc.vector.reciprocal(inv[:], pcs[:])
                                pbc = ps1b.tile([P, 512], f32, tag="pbc2")
                                nc.tensor.matmul(pbc[:], ones1r[:], inv[:],
                                                 start=True, stop=True)
                                binv = p1.tile([HD, 512], f32, tag="binv")
                                nc.vector.tensor_copy(binv[:], pbc[:HD, :])
                                nc.vector.tensor_tensor(
                                    attn_sb[:, h, qt * 512:(qt + 1) * 512],
                                    ppv[:], binv[:], Alu.mult)

                    # feature->token AllToAll: block b = my 128 attn features
                    # for token tile b
                    for b in range(KT):
                        nc.sync.dma_start(
                            a2a_in[b, :, :].rearrange("(h d) t -> d h t", h=2),
                            attn_sb[:, :, b * P:(b + 1) * P].bitcast(f32))
                    if not profile and not _os.environ.get("NOA2A"):
                        nc.gpsimd.collective_compute(
                            "AllToAll", Alu.bypass, replica_groups=rg,
                            ins=[a2a_in[:]], outs=[a2a_out[:]])
                    elif not profile:
                        nc.sync.dma_start(a2a_out[:], a2a_in[:])
            # hT pool closed here

            # =========== phase 2: token-sliced o-proj + resid + ln2 + router =
            w2pool_cm = tc.tile_pool(name="w2pool", bufs=1)
            w2pool = w2pool_cm.__enter__()
            w2tp = w2pool.tile([P, JT, H], bf16)
            nc.sync.dma_start(w2tp[:], w2tp_d[:])

            h2f = pers.tile([P, H], f32)
            wtm_pack = pers.tile([P, 32], f32)
            with tc.tile_pool(name="p2", bufs=1) as p2, \
                 tc.tile_pool(name="ps2", bufs=1, space="PSUM") as ps2:
                owT = p2.tile([P, KT, H], f32r)
                nc.sync.dma_start(owT[:], owTf_d[:])
                af = p2.tile([P, KT, P], f32r)
                src = a2a_in if profile else a2a_out
                nc.sync.dma_start(
                    af[:], src[:].rearrange("k f t -> f k t").bitcast(f32r))

                resid = p2.tile([P, H], f32)
                for ni in range(2):
                    po = ps2.tile([P, 512], f32, tag="po", bufs=2)
                    for k in range(KT):
                        nc.tensor.matmul(po[:], af[:, k, :],
                                         owT[:, k, ni * 512:(ni + 1) * 512],
                                         start=(k == 0), stop=(k == KT - 1))
                    nc.vector.tensor_tensor(resid[:, ni * 512:(ni + 1) * 512],
                                            po[:],
                                            htok[:, ni * 512:(ni + 1) * 512],
                                            Alu.add)
                nc.sync.dma_start(resid_sl_d[:], resid[:])

                # token-major rmsnorm (ln2) for this 128-token slice
                sq2 = p2.tile([P, H], f32)
                nc.scalar.activation(sq2[:], resid[:], Act.Square)
                var = p2.tile([P, 1], f32)
                nc.vector.reduce_sum(var[:], sq2[:], axis=AxX)
                v2 = p2.tile([P, 1], f32)
                nc.vector.tensor_scalar(v2[:], var[:], 1.0 / H, EPS,
                                        Alu.mult, Alu.add)
                sd2 = p2.tile([P, 1], f32)
                nc.scalar.activation(sd2[:], v2[:], Act.Sqrt)
                inv2 = p2.tile([P, 1], f32)
                nc.vector.reciprocal(inv2[:], sd2[:])
                nc.vector.scalar_tensor_tensor(h2f[:], resid[:], inv2[:],
                                               ln2bc[:], Alu.mult, Alu.mult)

                # h2 (bf16) to DRAM for the expert gather
                h2bf = p2.tile([P, H], bf16)
                nc.vector.tensor_copy(h2bf[:], h2f[:])
                nc.sync.dma_start(ag2a_in[:], h2bf[:])

                # router on own 128 tokens: transpose h2 -> feature-major
                h2T_sl = p2.tile([P, KT, P], f32)
                for k in range(KT):
                    ptk = ps2.tile([P, P], f32, tag="ptk", bufs=2)
                    nc.tensor.transpose(
                        ptk[:], h2f[:, k * P:(k + 1) * P], ident[:])
                    nc.vector.tensor_copy(h2T_sl[:, k, :], ptk[:])

                plog = ps2.tile([E, P], f32, tag="plog")
                for k in range(KT):
                    nc.tensor.matmul(plog[:], rwT[:, k, :], h2T_sl[:, k, :],
                                     start=(k == 0), stop=(k == KT - 1))
                logsb = p2.tile([E, P], f32)
                nc.vector.tensor_copy(logsb[:], plog[:])
                ptr = ps2.tile([P, E], f32, tag="ptr")
                nc.tensor.transpose(ptr[:], logsb[:], ident[:E, :E])
                lg = p2.tile([P, E], f32)
                nc.vector.tensor_copy(lg[:], ptr[:])

                # softmax + top-2 (f32, selection-exact)
                m1 = p2.tile([P, 1], f32)
                nc.vector.reduce_max(m1[:], lg[:], axis=AxX)
                nm1 = p2.tile([P, 1], f32)
                nc.vector.tensor_scalar_mul(nm1[:], m1[:], -1.0)
                ex = p2.tile([P, E], f32)
                nc.scalar.activation(ex[:], lg[:], Act.Exp, bias=nm1[:])
                den = p2.tile([P, 1], f32)
                nc.vector.reduce_sum(den[:], ex[:], axis=AxX)
                dinv = p2.tile([P, 1], f32)
                nc.vector.reciprocal(dinv[:], den[:])
                probs = p2.tile([P, E], f32)
                nc.vector.tensor_scalar_mul(probs[:], ex[:], dinv[:])

                w1 = p2.tile([P, 1], f32)
                nc.vector.reduce_max(w1[:], probs[:], axis=AxX)
                eq1 = p2.tile([P, E], f32)
                nc.vector.tensor_scalar(eq1[:], probs[:], w1[:], None,
                                        Alu.is_ge)
                it1 = p2.tile([P, E], f32)
                nc.vector.tensor_tensor(it1[:], eq1[:], iotaT[:, :E], Alu.mult)
                i1 = p2.tile([P, 1], f32)
                nc.vector.reduce_max(i1[:], it1[:], axis=AxX)
                pm = p2.tile([P, E], f32)
                nc.vector.tensor_tensor(pm[:], probs[:], eq1[:], Alu.mult)
                masked = p2.tile([P, E], f32)
                nc.vector.tensor_tensor(masked[:], probs[:], pm[:],
                                        Alu.subtract)
                w2 = p2.tile([P, 1], f32)
                nc.vector.reduce_max(w2[:], masked[:], axis=AxX)
                eq2 = p2.tile([P, E], f32)
                nc.vector.tensor_scalar(eq2[:], masked[:], w2[:], None,
                                        Alu.is_ge)
                it2 = p2.tile([P, E], f32)
                nc.vector.tensor_tensor(it2[:], eq2[:], iotaT[:, :E], Alu.mult)
                i2 = p2.tile([P, 1], f32)
                nc.vector.reduce_max(i2[:], it2[:], axis=AxX)

                nc.vector.memset(wtm_pack[:, 4:32], 0.0)
                nc.vector.tensor_copy(wtm_pack[:, 0:1], w1[:])
                nc.vector.tensor_copy(wtm_pack[:, 1:2], w2[:])
                nc.vector.tensor_copy(wtm_pack[:, 2:3], i1[:])
                nc.vector.tensor_copy(wtm_pack[:, 3:4], i2[:])
                nc.sync.dma_start(ag2b_in[:], wtm_pack[:])
                nc.sync.dma_start(dbg_wtm_d[:], wtm_pack[:, 0:4])

                if not profile and not _os.environ.get("NOAGB"):
                    nc.gpsimd.collective_compute(
                        "AllGather", Alu.bypass, replica_groups=rg,
                        ins=[ag2b_in[:]], outs=[ag2b_out[:]])
                elif not profile:
                    nc.sync.dma_start(ag2b_out[0:P, :], ag2b_in[:])
                if not profile and not _os.environ.get("NOAGA"):
                    nc.gpsimd.collective_compute(
                        "AllGather", Alu.bypass, replica_groups=rg,
                        ins=[ag2a_in[:]], outs=[ag2a_out[:]])
                elif not profile:
                    nc.sync.dma_start(ag2a_out[0:P, :], ag2a_in[:])

            # =========== phase 3: routing index build for my expert ==========
            moepool_cm = tc.tile_pool(name="moepool", bufs=1)
            moepool = moepool_cm.__enter__()
            gat = moepool.tile([P, RC], f32)     # per-slot combine weight
            S0 = moepool.tile([P, RC, T], bf16)  # per-slot 0/1 scatter rows
            idxs_sb = moepool.tile([P, C // 16], i16)
            h2c = moepool.tile([P, KT, C], bf16)

            with tc.tile_pool(name="p3", bufs=1) as p3, \
                 tc.tile_pool(name="ps3", bufs=1, space="PSUM") as ps3:
                wtm_src = ag2b_in if profile else ag2b_out
                wtm_sb = p3.tile([P, KT, 4], f32)
                nc.sync.dma_start(
                    wtm_sb[:],
                    wtm_src[:, 0:4].rearrange("(ti p) k -> p ti k", p=P))

                # wcol[t] = weight of my expert for token t (0 if unselected)
                eqa = p3.tile([P, KT], f32)
                nc.vector.tensor_scalar(eqa[:], wtm_sb[:, :, 2], colc[:], None,
                                        Alu.is_equal)
                wa = p3.tile([P, KT], f32)
                nc.vector.tensor_tensor(wa[:], eqa[:], wtm_sb[:, :, 0],
                                        Alu.mult)
                eqb = p3.tile([P, KT], f32)
                nc.vector.tensor_scalar(eqb[:], wtm_sb[:, :, 3], colc[:], None,
                                        Alu.is_equal)
                wb_ = p3.tile([P, KT], f32)
                nc.vector.tensor_tensor(wb_[:], eqb[:], wtm_sb[:, :, 1],
                                        Alu.mult)
                wcol = p3.tile([P, KT], f32r)
                nc.vector.tensor_tensor(wcol[:], wa[:], wb_[:], Alu.add)
                ind = p3.tile([P, KT], f32)
                nc.vector.tensor_scalar(ind[:], wcol[:], 0.0, None, Alu.is_gt)
                ind_r = p3.tile([P, KT], f32r)
                nc.vector.tensor_copy(ind_r[:], ind[:])

                # counts per tile -> exclusive offsets (row form)
                pcnt = ps3.tile([KT, 2], f32, tag="pcnt")
                nc.tensor.matmul(pcnt[:], ind_r[:], ones2[:],
                                 start=True, stop=True)
                cnts = p3.tile([KT, 1], f32r)
                nc.vector.tensor_copy(cnts[:], pcnt[:, 0:1])
                # total count (for host-side capacity check)
                ptot = ps3.tile([1, 2], f32, tag="ptot")
                nc.tensor.matmul(ptot[:], cnts[:], ones2[0:KT, :],
                                 start=True, stop=True)
                ctot = p3.tile([1, 1], f32)
                nc.vector.tensor_copy(ctot[:], ptot[:, 0:1])
                nc.sync.dma_start(cnt_d[:], ctot[:])

                poff = ps3.tile([1, KT], f32, tag="poff")
                nc.tensor.matmul(poff[:], cnts[:], u8[:],
                                 start=True, stop=True)
                offsrow = p3.tile([1, KT], f32r)
                nc.vector.tensor_copy(offsrow[:], poff[:])

                # global rank of each token within my expert's list
                prank = ps3.tile([P, KT], f32, tag="prank")
                nc.tensor.matmul(prank[:], l128[:], ind_r[:],
                                 start=True, stop=False)
                nc.tensor.matmul(prank[:], ones1r[:], offsrow[:],
                                 start=False, stop=True)
                grank = p3.tile([P, KT], f32)
                nc.vector.tensor_copy(grank[:], prank[:])

                # M matrices + pos list + per-slot weights
                wcol2 = p3.tile([P, KT, 2], f32r)
                nc.vector.tensor_copy(wcol2[:, :, 0], wcol[:])
                nc.vector.tensor_copy(wcol2[:, :, 1], wcol[:])
                M8 = p3.tile([P, KT, C], f32r)
                for ti in range(KT):
                    nc.vector.tensor_scalar(M8[:, ti, :], iotaT[:, :C],
                                            grank[:, ti:ti + 1],
                                            ind[:, ti:ti + 1],
                                            Alu.is_equal, Alu.mult)
                ppos = ps3.tile([1, C], f32, tag="ppos")
                for ti in range(KT):
                    nc.tensor.matmul(ppos[:], tokidx[:, ti:ti + 1],
                                     M8[:, ti, :],
                                     start=(ti == 0), stop=(ti == KT - 1))
                pwsl = ps3.tile([P, RC, 2], f32, tag="pwsl")
                for ch in range(RC):
                    for ti in range(KT):
                        nc.tensor.matmul(pwsl[:, ch, :],
                                         M8[:, ti, ch * P:(ch + 1) * P],
                                         wcol2[:, ti, :],
                                         start=(ti == 0), stop=(ti == KT - 1))
                nc.vector.tensor_copy(gat[:], pwsl[:, :, 0])

                pos_sb = p3.tile([1, C], f32r)
                nc.vector.tensor_copy(pos_sb[:], ppos[:])
                nc.sync.dma_start(dbg_pos_d[:], pos_sb[:].bitcast(f32))
                nc.sync.dma_start(dbg_gat_d[:], gat[:])
                pos_i16 = p3.tile([1, C], i16)
                nc.vector.tensor_copy(pos_i16[:], pos_sb[:])
                nc.sync.dma_start(idx_d[:], pos_i16[:])
                # wrap to [16, C/16] and replicate to all 128 partitions
                nc.sync.dma_start(idxs_sb[0:16, :],
                                  idx_d[0, :].rearrange("(s p) -> p s", p=16))
                nc.sync.dma_start(idxs_sb[16:32, :], idxs_sb[0:16, :])
                nc.sync.dma_start(idxs_sb[32:64, :], idxs_sb[0:32, :])
                nc.sync.dma_start(idxs_sb[64:128, :], idxs_sb[0:64, :])

                # slot position columns -> S0 scatter rows
                posch = p3.tile([P, RC], f32)
                for ch in range(RC):
                    ptp = ps3.tile([P, 1], f32, tag="ptp", bufs=2)
                    nc.tensor.transpose(
                        ptp[:], pos_sb[0:1, ch * P:(ch + 1) * P].bitcast(f32),
                        ident[0:1, 0:1])
                    nc.vector.tensor_copy(posch[:, ch:ch + 1], ptp[:])
                for ch in range(RC):
                    nc.vector.tensor_scalar(S0[:, ch, :], iotaT[:],
                                            posch[:, ch:ch + 1], None,
                                            Alu.is_equal)

                # gather my expert's tokens (bf16, feature-major)
                h2src = ag2a_in if profile else ag2a_out
                if _os.environ.get("NOGATHER"):
                    for k in range(KT):
                        nc.sync.dma_start(h2c[:, k, :],
                                          (h2src if not profile
                                           else ag2a_in)[0:P, 0:C])
                else:
                    nc.gpsimd.dma_gather(
                        h2c[:], h2src[:], idxs_sb[:], C, C, H,
                        transpose=True)

            with tc.tile_pool(name="dbgp", bufs=1) as dbgp:
                dtmp = dbgp.tile([P, KT, 8], f32)
                for k in range(KT):
                    nc.vector.tensor_copy(dtmp[:, k, :], h2c[:, k, 0:8])
                nc.sync.dma_start(dbg_h2c_d[:], dtmp[:])

            # =========== phase 4: expert FFN on C tokens =====================
            with tc.tile_pool(name="wpool", bufs=2) as wpool, \
                 tc.tile_pool(name="apool", bufs=1) as apool, \
                 tc.tile_pool(name="spool", bufs=2) as spool, \
                 tc.tile_pool(name="ps4", bufs=1, space="PSUM") as ps4:
                act = apool.tile([P, JT, C], bf16)
                for j in range(JT):
                    wg = wpool.tile([P, KT * P], bf16, tag="wg", bufs=2)
                    nc.sync.dma_start(wg[:], wspb_d[j])
                    wu = wpool.tile([P, KT * P], bf16, tag="wu", bufs=2)
                    nc.sync.dma_start(wu[:], wspb_d[JT + j])
                    pg = ps4.tile([P, C], f32, tag="pg", bufs=2)
                    pu = ps4.tile([P, C], f32, tag="pu", bufs=2)
                    for k in range(KT):
                        nc.tensor.matmul(pg[:], wg[:, k * P:(k + 1) * P],
                                         h2c[:, k, :],
                                         start=(k == 0), stop=(k == KT - 1))
                    for k in range(KT):
                        nc.tensor.matmul(pu[:], wu[:, k * P:(k + 1) * P],
                                         h2c[:, k, :],
                                         start=(k == 0), stop=(k == KT - 1))
                    sil = spool.tile([P, C], f32, tag="sil")
                    nc.scalar.activation(sil[:], pg[:], Act.Silu)
                    nc.vector.tensor_tensor(act[:, j, :], sil[:], pu[:],
                                            Alu.mult)

                # down proj, slot-major output [slots, H], scaled by gatings
                cmp_bf = apool.tile([P, RC, H], bf16)
                for ch in range(RC):
                    for ni in range(2):
                        pd = ps4.tile([P, 512], f32, tag="pd", bufs=2)
                        for j in range(JT):
                            nc.tensor.matmul(
                                pd[:], act[:, j, ch * P:(ch + 1) * P],
                                w2tp[:, j, ni * 512:(ni + 1) * 512],
                                start=(j == 0), stop=(j == JT - 1))
                        nc.vector.tensor_scalar(
                            cmp_bf[:, ch, ni * 512:(ni + 1) * 512], pd[:],
                            gat[:, ch:ch + 1], None, Alu.mult)

                # scatter back to dense [T, H] via 0/1 scatter-matmul
                for tt in range(KT):
                    for ni in range(2):
                        psc = ps4.tile([P, 512], f32, tag="psc", bufs=2)
                        for ch in range(RC):
                            nc.tensor.matmul(
                                psc[:], S0[:, ch, tt * P:(tt + 1) * P],
                                cmp_bf[:, ch, ni * 512:(ni + 1) * 512],
                                start=(ch == 0), stop=(ch == RC - 1))
                        eo = spool.tile([P, 512], bf16, tag="eo")
                        nc.vector.tensor_copy(eo[:], psc[:])
                        nc.sync.dma_start(
                            rs_in[tt * P:(tt + 1) * P,
                                  ni * 512:(ni + 1) * 512], eo[:])

                if not profile and not _os.environ.get("NORS"):
                    nc.gpsimd.collective_compute(
                        "ReduceScatter", Alu.add, replica_groups=rg,
                        ins=[rs_in[:]], outs=[rs_out[:]])
                elif not profile:
                    nc.sync.dma_start(rs_out[:], rs_in[0:P, :])
                out_sb = spool.tile([P, H], bf16, tag="osb")
                nc.sync.dma_start(out_sb[:],
                                  rs_out[:] if not profile else rs_in[0:P, :])
                nc.sync.dma_start(moe_sl_d[:], out_sb[:])

            moepool_cm.__exit__(None, None, None)
            w2pool_cm.__exit__(None, None, None)

    nc.compile()
    return nc


_NC = None


def _get_nc():
    global _NC
    if _NC is None:
        _NC = _build()
    return _NC


def _pack_inputs(hidden_states, ln1_w, qkv_w, o_w, ln2_w, router_w, ws, w2s):
    import ml_dtypes
    hidden_states = np.asarray(hidden_states, np.float32)
    qkv_w = np.asarray(qkv_w, np.float32)
    o_w = np.asarray(o_w, np.float32)
    router_w = np.asarray(router_w, np.float32)
    ws = np.asarray(ws, np.float32)
    w2s = np.asarray(w2s, np.float32)
    ln1_w = np.asarray(ln1_w, np.float32)
    ln2_w = np.asarray(ln2_w, np.float32)

    hT = np.ascontiguousarray(hidden_states.T)
    ln1p = np.ascontiguousarray(ln1_w.reshape(KT, P).T)
    ln2row = np.ascontiguousarray(ln2_w.reshape(1, H))
    rwT = np.ascontiguousarray(router_w.T.reshape(KT, P, E).transpose(1, 0, 2))
    owTf = np.ascontiguousarray(
        o_w.T.reshape(KT, P, H).transpose(1, 0, 2))

    amask = np.empty((len(OFFS), P, 512), np.float32)
    pp = np.arange(P)[:, None]
    ff = np.arange(512)[None, :]
    for i, off in enumerate(OFFS):
        d = off + ff - pp
        amask[i] = np.where((d >= 0) & (d < SW), 0.0, NEG)
    amask = amask.astype(ml_dtypes.bfloat16)

    ones128 = np.ones((P, 1), np.float32)
    ones1r = np.ones((1, P), np.float32)
    ones2 = np.ones((P, 2), np.float32)
    iota_row = np.arange(T, dtype=np.float32).reshape(1, T)
    tokidx = (np.arange(P)[:, None] + 128 * np.arange(KT)[None, :]) \
        .astype(np.float32)
    l128 = (np.arange(P)[:, None] < np.arange(P)[None, :]).astype(np.float32)
    u8 = (np.arange(E)[:, None] < np.arange(E)[None, :]).astype(np.float32)

    in_maps = []
    for c in range(NCORES):
        qrows = qkv_w[2 * c * HD:(2 * c + 2) * HD]
        krows = qkv_w[NH * HD + (c // 2) * HD: NH * HD + (c // 2 + 1) * HD]
        vrows = qkv_w[(NH + NKV) * HD + (c // 2) * HD:
                      (NH + NKV) * HD + (c // 2 + 1) * HD]
        qkv_sh = np.concatenate([qrows, krows, vrows], axis=0)   # [256, H]
        qkvwT = np.ascontiguousarray(qkv_sh.T)                   # [H, 256]

        wsT = ws[c].T                                            # [H, 2I]
        wspb = np.ascontiguousarray(
            wsT.reshape(KT, P, 2 * JT, P).transpose(2, 1, 0, 3)
               .reshape(2 * JT, P, KT * P)).astype(ml_dtypes.bfloat16)
        w2T = w2s[c].T                                           # [I, H]
        w2tp = np.ascontiguousarray(
            w2T.reshape(JT, P, H).transpose(1, 0, 2)).astype(ml_dtypes.bfloat16)

        htok = np.ascontiguousarray(hidden_states[c * P:(c + 1) * P, :])
        colcv = np.full((P, 1), float(c), np.float32)

        in_maps.append({
            "hT": hT, "htok": htok, "qkvwT": qkvwT, "owTf": owTf,
            "ln1w": ln1p, "ln2row": ln2row, "rwT": rwT,
            "wspb": wspb, "w2tp": w2tp, "amask": amask,
            "iota_row": iota_row, "ones128": ones128, "ones1r": ones1r,
            "ones2": ones2, "colc": colcv, "tokidx": tokidx,
            "l128": l128, "u8": u8,
        })
    return in_maps


def _host_reference(hidden_states, ln1_w, qkv_w, o_w, ln2_w, router_w, ws, w2s):
    """Numpy fallback (only used if an expert exceeds the 384-token capacity,
    which cannot happen for headroom-style inputs; kept for safety)."""
    x = np.asarray(hidden_states, np.float32)

    def rms(v, w):
        var = (v * v).mean(-1, keepdims=True)
        return v / np.sqrt(var + EPS) * w

    h = rms(x, ln1_w)
    qkv = h @ qkv_w.T
    q = qkv[:, :NH * HD].reshape(T, NH, HD)
    k = qkv[:, NH * HD:(NH + NKV) * HD].reshape(T, NKV, HD)
    v = qkv[:, (NH + NKV) * HD:].reshape(T, NKV, HD)
    rep = NH // NKV
    k = np.repeat(k, rep, axis=1)
    v = np.repeat(v, rep, axis=1)
    sc = np.einsum('qhd,khd->hqk', q, k) * SCALE
    ii = np.arange(T)[:, None]
    jj = np.arange(T)[None, :]
    mask = (jj <= ii) & ((ii - jj) < SW)
    sc = np.where(mask[None], sc, NEG)
    sc -= sc.max(-1, keepdims=True)
    p = np.exp(sc)
    p /= p.sum(-1, keepdims=True)
    attn = np.einsum('hqk,khd->qhd', p, v).reshape(T, NH * HD)
    resid = x + attn @ o_w.T
    h2 = rms(resid, ln2_w)
    logits = h2 @ router_w.T
    lm = logits.max(-1, keepdims=True)
    pe = np.exp(logits - lm)
    probs = pe / pe.sum(-1, keepdims=True)
    order = np.argsort(-probs, axis=1)[:, :2]
    moe = np.zeros((T, H), np.float32)
    for e in range(E):
        sel = (order == e).any(axis=1)
        wsel = np.where(sel, probs[:, e], 0.0)
        gu = h2 @ ws[e].T
        g, u = gu[:, :I], gu[:, I:]
        a = (g / (1.0 + np.exp(-g))) * u
        moe += wsel[:, None] * (a @ w2s[e].T)
    return moe, resid


def kernel(hidden_states, positions, ln1_w, qkv_w, o_w, ln2_w, router_w, ws, w2s):
    nc = _get_nc()
    in_maps = _pack_inputs(hidden_states, ln1_w, qkv_w, o_w, ln2_w,
                           router_w, ws, w2s)
    res = run_bass_kernel_spmd(nc, in_maps, list(range(NCORES)))
    counts = [float(res.results[c]["cnt"][0, 0]) for c in range(NCORES)]
    if max(counts) > C:
        return _host_reference(hidden_states, ln1_w, qkv_w, o_w, ln2_w,
                               router_w, ws, w2s)
    moe_out = np.concatenate(
        [np.asarray(res.results[c]["moe_sl"], np.float32)
         for c in range(NCORES)], axis=0)                        # [T, H]
    residual = np.concatenate(
        [np.asarray(res.results[c]["resid_sl"], np.float32)
         for c in range(NCORES)], axis=0)                        # [T, H]
    return moe_out, residual


# revision 46
# speedup vs baseline: 14436.4846x; 13443.4776x over previous
"""Trainium2 Bass kernel for nn_JambaAttentionDecoderLayer (8-core SPMD).

v2: routed MoE + restructured collectives.

Sharding: tensor-parallel attention (2 q-heads + 1 kv-head per core) with an
AllToAll feature->token exchange, token-sliced o-proj/rmsnorm/router
(128 tokens per core), and expert parallelism for the MoE with true top-2
token routing: per-expert token index lists are built on device with
matmul-based stream compaction, tokens gathered with the DGE dma_gather
(transpose mode, bf16), expert FFN runs on a fixed 384-token capacity
(vs 1024 dense), and results are scattered back with a 0/1 scatter-matmul
followed by a bf16 ReduceScatter.

Precision: the attention -> residual -> rmsnorm -> router-logits path stays
in f32/f32r so the top-2 expert *selection* matches the f32 reference
(min |logit2-logit3| gap for this model is ~5e-4; bf16 noise would flip
experts). Everything downstream of selection (expert inputs, weights,
combine) runs in bf16: errors there scale with the element magnitude and
stay well inside the 2e-2 gate.
"""

import os as _os
import numpy as np

import concourse.bass as bass
import concourse.tile as tile
import concourse.mybir as mybir
from concourse import bacc
from concourse.bass_utils import run_bass_kernel_spmd

# dims (hardcoded per spec)
T = 1024
H = 1024
NH = 16
NKV = 4
HD = 64
I = 2816
E = 8
SW = 512
EPS = 1e-6
SCALE = HD ** -0.5

NCORES = 8
P = 128
KT = H // P          # 8 k-tiles over H
JT = I // P          # 22 j-tiles over I
C = 384              # MoE token capacity per expert (max count here is 275)
RC = C // P          # 3 slot chunks
NEG = -1.0e30

f32 = mybir.dt.float32
f32r = mybir.dt.float32r
bf16 = mybir.dt.bfloat16
i16 = mybir.dt.int16

# attention mask offsets: off = q_tile_start - k_tile_start for [128k,512q] tiles
OFFS = [-384, -256, -128, 0, 128, 256, 384, 512]
QT_KIS = {0: list(range(0, 4)), 1: list(range(0, 8))}

AxX = mybir.AxisListType.X
Alu = mybir.AluOpType
Act = mybir.ActivationFunctionType


def _build(profile=False):
    ndev = 1 if profile else NCORES
    nc = bacc.Bacc("TRN2", target_bir_lowering=False, debug=False,
                   num_devices=ndev)

    # ---- kernel I/O ----
    hT_d = nc.dram_tensor("hT", [H, T], f32, kind="ExternalInput")
    htok_d = nc.dram_tensor("htok", [P, H], f32, kind="ExternalInput")
    qkvwT_d = nc.dram_tensor("qkvwT", [H, 256], f32r, kind="ExternalInput")
    owTf_d = nc.dram_tensor("owTf", [P, KT, H], f32r, kind="ExternalInput")
    ln1w_d = nc.dram_tensor("ln1w", [P, KT], f32, kind="ExternalInput")
    ln2row_d = nc.dram_tensor("ln2row", [1, H], f32r, kind="ExternalInput")
    rwT_d = nc.dram_tensor("rwT", [P, KT, E], f32, kind="ExternalInput")
    wspb_d = nc.dram_tensor("wspb", [2 * JT, P, KT * P], bf16,
                            kind="ExternalInput")
    w2tp_d = nc.dram_tensor("w2tp", [P, JT, H], bf16, kind="ExternalInput")
    amask_d = nc.dram_tensor("amask", [len(OFFS), P, 512], bf16,
                             kind="ExternalInput")
    iota_row_d = nc.dram_tensor("iota_row", [1, T], f32r, kind="ExternalInput")
    ones128_d = nc.dram_tensor("ones128", [P, 1], f32r, kind="ExternalInput")
    ones1r_d = nc.dram_tensor("ones1r", [1, P], f32r, kind="ExternalInput")
    colc_d = nc.dram_tensor("colc", [P, 1], f32, kind="ExternalInput")
    tokidx_d = nc.dram_tensor("tokidx", [P, KT], f32r, kind="ExternalInput")
    l128_d = nc.dram_tensor("l128", [P, P], f32r, kind="ExternalInput")
    u8_d = nc.dram_tensor("u8", [E, E], f32r, kind="ExternalInput")
    ones2_d = nc.dram_tensor("ones2", [P, 2], f32r, kind="ExternalInput")

    resid_sl_d = nc.dram_tensor("resid_sl", [P, H], f32, kind="ExternalOutput")
    moe_sl_d = nc.dram_tensor("moe_sl", [P, H], bf16, kind="ExternalOutput")
    cnt_d = nc.dram_tensor("cnt", [1, 1], f32, kind="ExternalOutput")

    rg = [list(range(NCORES))]

    import contextlib
    lp = getattr(nc, "allow_low_precision", None)
    lp_cm = lp(reason="f32r/bf16 matmul operands; within rel-err budget") \
        if lp else contextlib.nullcontext()
    with lp_cm, tile.TileContext(nc) as tc:
        with tc.tile_pool(name="const", bufs=1) as cpool, \
             tc.tile_pool(name="persist", bufs=1) as pers, \
             tc.tile_pool(name="dram", bufs=1, space="DRAM") as dram:

            # ---- constants ----
            ones128 = cpool.tile([P, 1], f32r)
            nc.sync.dma_start(ones128[:], ones128_d[:])
            ones1r = cpool.tile([1, P], f32r)
            nc.sync.dma_start(ones1r[:], ones1r_d[:])
            ln1w = cpool.tile([P, KT], f32)
            nc.sync.dma_start(ln1w[:], ln1w_d[:])
            colc = cpool.tile([P, 1], f32)
            nc.sync.dma_start(colc[:], colc_d[:])
            tokidx = cpool.tile([P, KT], f32r)
            nc.sync.dma_start(tokidx[:], tokidx_d[:])
            l128 = cpool.tile([P, P], f32r)
            nc.sync.dma_start(l128[:], l128_d[:])
            u8 = cpool.tile([E, E], f32r)
            nc.sync.dma_start(u8[:], u8_d[:])
            ones2 = cpool.tile([P, 2], f32r)
            nc.sync.dma_start(ones2[:], ones2_d[:])
            ident = cpool.tile([P, P], f32)
            from concourse.masks import make_identity
            make_identity(nc, ident[:])
            iota_row = cpool.tile([1, T], f32r)
            nc.sync.dma_start(iota_row[:], iota_row_d[:])

            htok = pers.tile([P, H], f32)
            nc.sync.dma_start(htok[:], htok_d[:])
            rwT = pers.tile([P, KT, E], f32)
            nc.sync.dma_start(rwT[:], rwT_d[:])
            ln2row = pers.tile([1, H], f32r)
            nc.sync.dma_start(ln2row[:], ln2row_d[:])

            # broadcast rows: iota [128, T] and ln2w [128, H]
            iotaT = pers.tile([P, T], f32)
            ln2bc = pers.tile([P, H], f32)
            with tc.tile_pool(name="bc", bufs=1, space="PSUM") as bcps:
                for ni in range(2):
                    pbi = bcps.tile([P, 512], f32, tag="pbi", bufs=2)
                    nc.tensor.matmul(pbi[:], ones1r[:],
                                     iota_row[:, ni * 512:(ni + 1) * 512],
                                     start=True, stop=True)
                    nc.vector.tensor_copy(iotaT[:, ni * 512:(ni + 1) * 512],
                                          pbi[:])
                    pbl = bcps.tile([P, 512], f32, tag="pbl", bufs=2)
                    nc.tensor.matmul(pbl[:], ones1r[:],
                                     ln2row[:, ni * 512:(ni + 1) * 512],
                                     start=True, stop=True)
                    nc.vector.tensor_copy(ln2bc[:, ni * 512:(ni + 1) * 512],
                                          pbl[:])

            # dram bounce buffers for collectives
            a2a_in = dram.tile([KT, P, P], f32)
            a2a_out = dram.tile([KT, P, P], f32)
            ag2a_in = dram.tile([P, H], bf16)
            ag2a_out = dram.tile([T, H], bf16, addr_space="Shared")
            ag2b_in = dram.tile([P, 32], f32)
            ag2b_out = dram.tile([T, 32], f32, addr_space="Shared")
            idx_d = dram.tile([1, C], i16)
            rs_in = dram.tile([T, H], bf16)
            rs_out = dram.tile([P, H], bf16)

            # =========== feature-major RMSNorm (ln1), as baseline ===========
            def rmsnorm_fm(src_tile, lnw_tile, dst_tile):
                with tc.tile_pool(name="rn", bufs=1) as tmp, \
                     tc.tile_pool(name="rnps", bufs=1, space="PSUM") as psum:
                    vs = [None, None]
                    for ni in range(2):
                        pv = psum.tile([1, 512], f32, tag="pvar")
                        for k in range(KT):
                            sq = tmp.tile([P, 512], f32r, tag="sq", bufs=2)
                            nc.scalar.activation(
                                sq[:], src_tile[:, k, ni * 512:(ni + 1) * 512],
                                Act.Square)
                            nc.tensor.matmul(pv[:], ones128[:], sq[:],
                                             start=(k == 0), stop=(k == KT - 1))
                        v = tmp.tile([1, 512], f32, tag="vv")
                        nc.vector.tensor_scalar(v[:], pv[:], 1.0 / H, EPS,
                                                Alu.mult, Alu.add)
                        sd = tmp.tile([1, 512], f32, tag="sd")
                        nc.scalar.activation(sd[:], v[:], Act.Sqrt)
                        s = tmp.tile([1, 512], f32r, tag="ss")
                        nc.vector.reciprocal(s[:], sd[:])
                        pb = psum.tile([P, 512], f32, tag="pbc", bufs=2)
                        nc.tensor.matmul(pb[:], ones1r[:], s[:],
                                         start=True, stop=True)
                        vs[ni] = pb
                    for ni in range(2):
                        for k in range(KT):
                            nc.vector.scalar_tensor_tensor(
                                dst_tile[:, k, ni * 512:(ni + 1) * 512],
                                src_tile[:, k, ni * 512:(ni + 1) * 512],
                                lnw_tile[:, k:k + 1],
                                vs[ni][:],
                                Alu.mult, Alu.mult)

            # =========== phase 1: ln1 + qkv + attention (f32r, as baseline) ==
            with tc.tile_pool(name="hp", bufs=1) as hp:
                hT = hp.tile([P, KT, T], f32)
                nc.sync.dma_start(hT[:], hT_d.rearrange("(k p) t -> p k t", p=P))

                with tc.tile_pool(name="p1", bufs=2) as p1:
                    qkvT = p1.tile([P, 2, T], f32r, bufs=1)
                    with tc.tile_pool(name="p1a", bufs=1) as p1a:
                        hnT = p1a.tile([P, KT, T], f32r)
                        rmsnorm_fm(hT, ln1w, hnT)

                        with tc.tile_pool(name="ps1", bufs=1, space="PSUM") as ps1:
                            qkvw = p1a.tile([P, KT, 256], f32r)
                            nc.sync.dma_start(
                                qkvw[:], qkvwT_d.rearrange("(k p) m -> p k m", p=P))
                            for mi in range(2):
                                for ni in range(2):
                                    pq = ps1.tile([P, 512], f32, tag="pqkv", bufs=2)
                                    for k in range(KT):
                                        nc.tensor.matmul(
                                            pq[:], qkvw[:, k, mi * P:(mi + 1) * P],
                                            hnT[:, k, ni * 512:(ni + 1) * 512],
                                            start=(k == 0), stop=(k == KT - 1))
                                    nc.vector.tensor_copy(
                                        qkvT[:, mi, ni * 512:(ni + 1) * 512], pq[:])

                    # v to token-major [128tok, 8tiles, 64]
                    v_sb = p1.tile([P, KT, HD], f32r, bufs=1)
                    with tc.tile_pool(name="ps1v", bufs=1, space="PSUM") as ps1v:
                        for ti in range(KT):
                            pvt = ps1v.tile([P, HD], f32, tag="pvt", bufs=2)
                            nc.tensor.transpose(
                                pvt[:],
                                qkvT[HD:P, 1, ti * P:(ti + 1) * P].bitcast(f32),
                                ident[HD:P, HD:P])
                            nc.vector.tensor_copy(v_sb[:, ti, :], pvt[:])

                    attn_sb = p1.tile([HD, 2, T], f32r, bufs=1)
                    am = p1.tile([P, len(OFFS), 512], bf16, bufs=1)
                    nc.sync.dma_start(am[:], amask_d.rearrange("o p f -> p o f"))

                    # re-base head-1 q to partitions 0..63 (SBUF->SBUF DMA)
                    q1_sb = p1.tile([HD, T], f32r, bufs=1)
                    nc.sync.dma_start(q1_sb[:], qkvT[HD:P, 0, :])

                    with tc.tile_pool(name="ps1b", bufs=1, space="PSUM") as ps1b:
                        for h in range(2):
                            qT = qkvT[0:HD, 0, :] if h == 0 else q1_sb[:]
                            kTT = qkvT[0:HD, 1, :]
                            for qt in range(2):
                                kis = QT_KIS[qt]
                                ppv = ps1b.tile([HD, 512], f32, tag="ppv")
                                pcs = ps1b.tile([1, 512], f32, tag="pcs")
                                for idx, ki in enumerate(kis):
                                    pscore = ps1b.tile([P, 512], f32,
                                                       tag="pscore", bufs=2)
                                    nc.tensor.matmul(
                                        pscore[:], kTT[:, ki * P:(ki + 1) * P],
                                        qT[:, qt * 512:(qt + 1) * 512],
                                        start=True, stop=True)
                                    off_i = OFFS.index(qt * 512 - ki * P)
                                    sm = p1.tile([P, 512], f32, tag="sm")
                                    nc.vector.scalar_tensor_tensor(
                                        sm[:], pscore[:], SCALE,
                                        am[:, off_i, :], Alu.mult, Alu.add)
                                    pexp = p1.tile([P, 512], f32r, tag="pexp")
                                    nc.scalar.activation(pexp[:], sm[:], Act.Exp)
                                    nc.tensor.matmul(
                                        pcs[:], ones128[:], pexp[:],
                                        start=(idx == 0),
                                        stop=(idx == len(kis) - 1))
                                    nc.tensor.matmul(
                                        ppv[:], v_sb[:, ki, :], pexp[:],
                                        start=(idx == 0),
                                        stop=(idx == len(kis) - 1))
                                inv = p1.tile([1, 512], f32r, tag="inv")
                                nc.vector.reciprocal(inv[:], pcs[:])
                                pbc = ps1b.tile([P, 512], f32, tag="pbc2")
                                nc.tensor.matmul(pbc[:], ones1r[:], inv[:],
                                                 start=True, stop=True)
                                binv = p1.tile([HD, 512], f32, tag="binv")
                                nc.vector.tensor_copy(binv[:], pbc[:HD, :])
                                nc.vector.tensor_tensor(
                                    attn_sb[:, h, qt * 512:(qt + 1) * 512],
                                    ppv[:], binv[:], Alu.mult)

                    # feature->token AllToAll: block b = my 128 attn features
                    # for token tile b
                    for b in range(KT):
                        nc.sync.dma_start(
                            a2a_in[b, :, :].rearrange("(h d) t -> d h t", h=2),
                            attn_sb[:, :, b * P:(b + 1) * P].bitcast(f32))
                    if not profile and not _os.environ.get("NOA2A"):
                        nc.gpsimd.collective_compute(
                            "AllToAll", Alu.bypass, replica_groups=rg,
                            ins=[a2a_in[:]], outs=[a2a_out[:]])
                    elif not profile:
                        nc.sync.dma_start(a2a_out[:], a2a_in[:])
            # hT pool closed here

            # =========== phase 2: token-sliced o-proj + resid + ln2 + router =
            w2pool_cm = tc.tile_pool(name="w2pool", bufs=1)
            w2pool = w2pool_cm.__enter__()
            w2tp = w2pool.tile([P, JT, H], bf16)
            nc.sync.dma_start(w2tp[:], w2tp_d[:])

            h2f = pers.tile([P, H], f32)
            wtm_pack = pers.tile([P, 32], f32)
            with tc.tile_pool(name="p2", bufs=1) as p2, \
                 tc.tile_pool(name="ps2", bufs=1, space="PSUM") as ps2:
                owT = p2.tile([P, KT, H], f32r)
                nc.sync.dma_start(owT[:], owTf_d[:])
                af = p2.tile([P, KT, P], f32r)
                src = a2a_in if profile else a2a_out
                nc.sync.dma_start(
                    af[:], src[:].rearrange("k f t -> f k t").bitcast(f32r))

                resid = p2.tile([P, H], f32)
                for ni in range(2):
                    po = ps2.tile([P, 512], f32, tag="po", bufs=2)
                    for k in range(KT):
                        nc.tensor.matmul(po[:], af[:, k, :],
                                         owT[:, k, ni * 512:(ni + 1) * 512],
                                         start=(k == 0), stop=(k == KT - 1))
                    nc.vector.tensor_tensor(resid[:, ni * 512:(ni + 1) * 512],
                                            po[:],
                                            htok[:, ni * 512:(ni + 1) * 512],
                                            Alu.add)
                nc.sync.dma_start(resid_sl_d[:], resid[:])

                # token-major rmsnorm (ln2) for this 128-token slice
                sq2 = p2.tile([P, H], f32)
                nc.scalar.activation(sq2[:], resid[:], Act.Square)
                var = p2.tile([P, 1], f32)
                nc.vector.reduce_sum(var[:], sq2[:], axis=AxX)
                v2 = p2.tile([P, 1], f32)
                nc.vector.tensor_scalar(v2[:], var[:], 1.0 / H, EPS,
                                        Alu.mult, Alu.add)
                sd2 = p2.tile([P, 1], f32)
                nc.scalar.activation(sd2[:], v2[:], Act.Sqrt)
                inv2 = p2.tile([P, 1], f32)
                nc.vector.reciprocal(inv2[:], sd2[:])
                nc.vector.scalar_tensor_tensor(h2f[:], resid[:], inv2[:],
                                               ln2bc[:], Alu.mult, Alu.mult)

                # h2 (bf16) to DRAM for the expert gather
                h2bf = p2.tile([P, H], bf16)
                nc.vector.tensor_copy(h2bf[:], h2f[:])
                nc.sync.dma_start(ag2a_in[:], h2bf[:])

                # router on own 128 tokens: transpose h2 -> feature-major
                h2T_sl = p2.tile([P, KT, P], f32)
                for k in range(KT):
                    ptk = ps2.tile([P, P], f32, tag="ptk", bufs=2)
                    nc.tensor.transpose(
                        ptk[:], h2f[:, k * P:(k + 1) * P], ident[:])
                    nc.vector.tensor_copy(h2T_sl[:, k, :], ptk[:])

                plog = ps2.tile([E, P], f32, tag="plog")
                for k in range(KT):
                    nc.tensor.matmul(plog[:], rwT[:, k, :], h2T_sl[:, k, :],
                                     start=(k == 0), stop=(k == KT - 1))
                logsb = p2.tile([E, P], f32)
                nc.vector.tensor_copy(logsb[:], plog[:])
                ptr = ps2.tile([P, E], f32, tag="ptr")
                nc.tensor.transpose(ptr[:], logsb[:], ident[:E, :E])
                lg = p2.tile([P, E], f32)
                nc.vector.tensor_copy(lg[:], ptr[:])

                # softmax + top-2 (f32, selection-exact)
                m1 = p2.tile([P, 1], f32)
                nc.vector.reduce_max(m1[:], lg[:], axis=AxX)
                nm1 = p2.tile([P, 1], f32)
                nc.vector.tensor_scalar_mul(nm1[:], m1[:], -1.0)
                ex = p2.tile([P, E], f32)
                nc.scalar.activation(ex[:], lg[:], Act.Exp, bias=nm1[:])
                den = p2.tile([P, 1], f32)
                nc.vector.reduce_sum(den[:], ex[:], axis=AxX)
                dinv = p2.tile([P, 1], f32)
                nc.vector.reciprocal(dinv[:], den[:])
                probs = p2.tile([P, E], f32)
                nc.vector.tensor_scalar_mul(probs[:], ex[:], dinv[:])

                w1 = p2.tile([P, 1], f32)
                nc.vector.reduce_max(w1[:], probs[:], axis=AxX)
                eq1 = p2.tile([P, E], f32)
                nc.vector.tensor_scalar(eq1[:], probs[:], w1[:], None,
                                        Alu.is_ge)
                it1 = p2.tile([P, E], f32)
                nc.vector.tensor_tensor(it1[:], eq1[:], iotaT[:, :E], Alu.mult)
                i1 = p2.tile([P, 1], f32)
                nc.vector.reduce_max(i1[:], it1[:], axis=AxX)
                pm = p2.tile([P, E], f32)
                nc.vector.tensor_tensor(pm[:], probs[:], eq1[:], Alu.mult)
                masked = p2.tile([P, E], f32)
                nc.vector.tensor_tensor(masked[:], probs[:], pm[:],
                                        Alu.subtract)
                w2 = p2.tile([P, 1], f32)
                nc.vector.reduce_max(w2[:], masked[:], axis=AxX)
                eq2 = p2.tile([P, E], f32)
                nc.vector.tensor_scalar(eq2[:], masked[:], w2[:], None,
                                        Alu.is_ge)
                it2 = p2.tile([P, E], f32)
                nc.vector.tensor_tensor(it2[:], eq2[:], iotaT[:, :E], Alu.mult)
                i2 = p2.tile([P, 1], f32)
                nc.vector.reduce_max(i2[:], it2[:], axis=AxX)

                nc.vector.memset(wtm_pack[:, 4:32], 0.0)
                nc.vector.tensor_copy(wtm_pack[:, 0:1], w1[:])
                nc.vector.tensor_copy(wtm_pack[:, 1:2], w2[:])
                nc.vector.tensor_copy(wtm_pack[:, 2:3], i1[:])
                nc.vector.tensor_copy(wtm_pack[:, 3:4], i2[:])
                nc.sync.dma_start(ag2b_in[:], wtm_pack[:])

                if not profile and not _os.environ.get("NOAGB"):
                    nc.gpsimd.collective_compute(
                        "AllGather", Alu.bypass, replica_groups=rg,
                        ins=[ag2b_in[:]], outs=[ag2b_out[:]])
                elif not profile:
                    nc.sync.dma_start(ag2b_out[0:P, :], ag2b_in[:])
                if not profile and not _os.environ.get("NOAGA"):
                    nc.gpsimd.collective_compute(
                        "AllGather", Alu.bypass, replica_groups=rg,
                        ins=[ag2a_in[:]], outs=[ag2a_out[:]])
                elif not profile:
                    nc.sync.dma_start(ag2a_out[0:P, :], ag2a_in[:])

            # =========== phase 3: routing index build for my expert ==========
            moepool_cm = tc.tile_pool(name="moepool", bufs=1)
            moepool = moepool_cm.__enter__()
            gat = moepool.tile([P, RC], f32)     # per-slot combine weight
            S0 = moepool.tile([P, RC, T], bf16)  # per-slot 0/1 scatter rows
            idxs_sb = moepool.tile([P, C // 16], i16)
            h2c = moepool.tile([P, KT, C], bf16)

            with tc.tile_pool(name="p3", bufs=1) as p3, \
                 tc.tile_pool(name="ps3", bufs=1, space="PSUM") as ps3:
                wtm_src = ag2b_in if profile else ag2b_out
                wtm_sb = p3.tile([P, KT, 4], f32)
                nc.sync.dma_start(
                    wtm_sb[:],
                    wtm_src[:, 0:4].rearrange("(ti p) k -> p ti k", p=P))

                # wcol[t] = weight of my expert for token t (0 if unselected)
                eqa = p3.tile([P, KT], f32)
                nc.vector.tensor_scalar(eqa[:], wtm_sb[:, :, 2], colc[:], None,
                                        Alu.is_equal)
                wa = p3.tile([P, KT], f32)
                nc.vector.tensor_tensor(wa[:], eqa[:], wtm_sb[:, :, 0],
                                        Alu.mult)
                eqb = p3.tile([P, KT], f32)
                nc.vector.tensor_scalar(eqb[:], wtm_sb[:, :, 3], colc[:], None,
                                        Alu.is_equal)
                wb_ = p3.tile([P, KT], f32)
                nc.vector.tensor_tensor(wb_[:], eqb[:], wtm_sb[:, :, 1],
                                        Alu.mult)
                wcol = p3.tile([P, KT], f32r)
                nc.vector.tensor_tensor(wcol[:], wa[:], wb_[:], Alu.add)
                ind = p3.tile([P, KT], f32)
                nc.vector.tensor_scalar(ind[:], wcol[:], 0.0, None, Alu.is_gt)
                ind_r = p3.tile([P, KT], f32r)
                nc.vector.tensor_copy(ind_r[:], ind[:])

                # counts per tile -> exclusive offsets (row form)
                pcnt = ps3.tile([KT, 2], f32, tag="pcnt")
                nc.tensor.matmul(pcnt[:], ind_r[:], ones2[:],
                                 start=True, stop=True)
                cnts = p3.tile([KT, 1], f32r)
                nc.vector.tensor_copy(cnts[:], pcnt[:, 0:1])
                # total count (for host-side capacity check)
                ptot = ps3.tile([1, 2], f32, tag="ptot")
                nc.tensor.matmul(ptot[:], cnts[:], ones2[0:KT, :],
                                 start=True, stop=True)
                ctot = p3.tile([1, 1], f32)
                nc.vector.tensor_copy(ctot[:], ptot[:, 0:1])
                nc.sync.dma_start(cnt_d[:], ctot[:])

                poff = ps3.tile([1, KT], f32, tag="poff")
                nc.tensor.matmul(poff[:], cnts[:], u8[:],
                                 start=True, stop=True)
                offsrow = p3.tile([1, KT], f32r)
                nc.vector.tensor_copy(offsrow[:], poff[:])

                # global rank of each token within my expert's list
                prank = ps3.tile([P, KT], f32, tag="prank")
                nc.tensor.matmul(prank[:], l128[:], ind_r[:],
                                 start=True, stop=False)
                nc.tensor.matmul(prank[:], ones1r[:], offsrow[:],
                                 start=False, stop=True)
                grank = p3.tile([P, KT], f32)
                nc.vector.tensor_copy(grank[:], prank[:])

                # M matrices + pos list + per-slot weights
                wcol2 = p3.tile([P, KT, 2], f32r)
                nc.vector.tensor_copy(wcol2[:, :, 0], wcol[:])
                nc.vector.tensor_copy(wcol2[:, :, 1], wcol[:])
                M8 = p3.tile([P, KT, C], f32r)
                for ti in range(KT):
                    nc.vector.tensor_scalar(M8[:, ti, :], iotaT[:, :C],
                                            grank[:, ti:ti + 1],
                                            ind[:, ti:ti + 1],
                                            Alu.is_equal, Alu.mult)
                ppos = ps3.tile([1, C], f32, tag="ppos")
                for ti in range(KT):
                    nc.tensor.matmul(ppos[:], tokidx[:, ti:ti + 1],
                                     M8[:, ti, :],
                                     start=(ti == 0), stop=(ti == KT - 1))
                pwsl = ps3.tile([P, RC, 2], f32, tag="pwsl")
                for ch in range(RC):
                    for ti in range(KT):
                        nc.tensor.matmul(pwsl[:, ch, :],
                                         M8[:, ti, ch * P:(ch + 1) * P],
                                         wcol2[:, ti, :],
                                         start=(ti == 0), stop=(ti == KT - 1))
                nc.vector.tensor_copy(gat[:], pwsl[:, :, 0])

                pos_sb = p3.tile([1, C], f32r)
                nc.vector.tensor_copy(pos_sb[:], ppos[:])
                pos_i16 = p3.tile([1, C], i16)
                nc.vector.tensor_copy(pos_i16[:], pos_sb[:])
                nc.sync.dma_start(idx_d[:], pos_i16[:])
                # wrap to [16, C/16] and replicate to all 128 partitions
                nc.sync.dma_start(idxs_sb[0:16, :],
                                  idx_d[0, :].rearrange("(s p) -> p s", p=16))
                nc.sync.dma_start(idxs_sb[16:32, :], idxs_sb[0:16, :])
                nc.sync.dma_start(idxs_sb[32:64, :], idxs_sb[0:32, :])
                nc.sync.dma_start(idxs_sb[64:128, :], idxs_sb[0:64, :])

                # gather my expert's tokens (bf16, feature-major)
                h2src = ag2a_in if profile else ag2a_out
                nc.gpsimd.dma_gather(
                    h2c[:], h2src[:], idxs_sb[:], C, C, H,
                    transpose=True)

                # slot position columns -> S0 scatter rows
                posch = p3.tile([P, RC], f32)
                for ch in range(RC):
                    ptp = ps3.tile([P, 1], f32, tag="ptp", bufs=2)
                    nc.tensor.transpose(
                        ptp[:], pos_sb[0:1, ch * P:(ch + 1) * P].bitcast(f32),
                        ident[0:1, 0:1])
                    nc.vector.tensor_copy(posch[:, ch:ch + 1], ptp[:])
                for ch in range(RC):
                    nc.vector.tensor_scalar(S0[:, ch, :], iotaT[:],
                                            posch[:, ch:ch + 1], None,
                                            Alu.is_equal)

            # =========== phase 4: expert FFN on C tokens =====================
            with tc.tile_pool(name="wpool", bufs=2) as wpool, \
                 tc.tile_pool(name="apool", bufs=1) as apool, \
                 tc.tile_pool(name="spool", bufs=2) as spool, \
                 tc.tile_pool(name="ps4", bufs=1, space="PSUM") as ps4:
                act = apool.tile([P, JT, C], bf16)
                for j in range(JT):
                    wg = wpool.tile([P, KT * P], bf16, tag="wg", bufs=2)
                    nc.sync.dma_start(wg[:], wspb_d[j])
                    wu = wpool.tile([P, KT * P], bf16, tag="wu", bufs=2)
                    nc.sync.dma_start(wu[:], wspb_d[JT + j])
                    pg = ps4.tile([P, C], f32, tag="pg", bufs=2)
                    pu = ps4.tile([P, C], f32, tag="pu", bufs=2)
                    for k in range(KT):
                        nc.tensor.matmul(pg[:], wg[:, k * P:(k + 1) * P],
                                         h2c[:, k, :],
                                         start=(k == 0), stop=(k == KT - 1))
                    for k in range(KT):
                        nc.tensor.matmul(pu[:], wu[:, k * P:(k + 1) * P],
                                         h2c[:, k, :],
                                         start=(k == 0), stop=(k == KT - 1))
                    sil = spool.tile([P, C], f32, tag="sil")
                    nc.scalar.activation(sil[:], pg[:], Act.Silu)
                    nc.vector.tensor_tensor(act[:, j, :], sil[:], pu[:],
                                            Alu.mult)

                # down proj, slot-major output [slots, H], scaled by gatings
                cmp_bf = apool.tile([P, RC, H], bf16)
                for ch in range(RC):
                    for ni in range(2):
                        pd = ps4.tile([P, 512], f32, tag="pd", bufs=2)
                        for j in range(JT):
                            nc.tensor.matmul(
                                pd[:], act[:, j, ch * P:(ch + 1) * P],
                                w2tp[:, j, ni * 512:(ni + 1) * 512],
                                start=(j == 0), stop=(j == JT - 1))
                        nc.vector.tensor_scalar(
                            cmp_bf[:, ch, ni * 512:(ni + 1) * 512], pd[:],
                            gat[:, ch:ch + 1], None, Alu.mult)

                # scatter back to dense [T, H] via 0/1 scatter-matmul
                for tt in range(KT):
                    for ni in range(2):
                        psc = ps4.tile([P, 512], f32, tag="psc", bufs=2)
                        for ch in range(RC):
                            nc.tensor.matmul(
                                psc[:], S0[:, ch, tt * P:(tt + 1) * P],
                                cmp_bf[:, ch, ni * 512:(ni + 1) * 512],
                                start=(ch == 0), stop=(ch == RC - 1))
                        eo = spool.tile([P, 512], bf16, tag="eo")
                        nc.vector.tensor_copy(eo[:], psc[:])
                        nc.sync.dma_start(
                            rs_in[tt * P:(tt + 1) * P,
                                  ni * 512:(ni + 1) * 512], eo[:])

                if not profile and not _os.environ.get("NORS"):
                    nc.gpsimd.collective_compute(
                        "ReduceScatter", Alu.add, replica_groups=rg,
                        ins=[rs_in[:]], outs=[rs_out[:]])
                elif not profile:
                    nc.sync.dma_start(rs_out[:], rs_in[0:P, :])
                out_sb = spool.tile([P, H], bf16, tag="osb")
                nc.sync.dma_start(out_sb[:],
                                  rs_out[:] if not profile else rs_in[0:P, :])
                nc.sync.dma_start(moe_sl_d[:], out_sb[:])

            moepool_cm.__exit__(None, None, None)
            w2pool_cm.__exit__(None, None, None)

    nc.compile()
    return nc


_NC = None


def _get_nc():
    global _NC
    if _NC is None:
        _NC = _build()
    return _NC


def _pack_inputs(hidden_states, ln1_w, qkv_w, o_w, ln2_w, router_w, ws, w2s):
    import ml_dtypes
    hidden_states = np.asarray(hidden_states, np.float32)
    qkv_w = np.asarray(qkv_w, np.float32)
    o_w = np.asarray(o_w, np.float32)
    router_w = np.asarray(router_w, np.float32)
    ws = np.asarray(ws, np.float32)
    w2s = np.asarray(w2s, np.float32)
    ln1_w = np.asarray(ln1_w, np.float32)
    ln2_w = np.asarray(ln2_w, np.float32)

    hT = np.ascontiguousarray(hidden_states.T)
    ln1p = np.ascontiguousarray(ln1_w.reshape(KT, P).T)
    ln2row = np.ascontiguousarray(ln2_w.reshape(1, H))
    rwT = np.ascontiguousarray(router_w.T.reshape(KT, P, E).transpose(1, 0, 2))
    owTf = np.ascontiguousarray(
        o_w.T.reshape(KT, P, H).transpose(1, 0, 2))

    amask = np.empty((len(OFFS), P, 512), np.float32)
    pp = np.arange(P)[:, None]
    ff = np.arange(512)[None, :]
    for i, off in enumerate(OFFS):
        d = off + ff - pp
        amask[i] = np.where((d >= 0) & (d < SW), 0.0, NEG)
    amask = amask.astype(ml_dtypes.bfloat16)

    ones128 = np.ones((P, 1), np.float32)
    ones1r = np.ones((1, P), np.float32)
    ones2 = np.ones((P, 2), np.float32)
    iota_row = np.arange(T, dtype=np.float32).reshape(1, T)
    tokidx = (np.arange(P)[:, None] + 128 * np.arange(KT)[None, :]) \
        .astype(np.float32)
    l128 = (np.arange(P)[:, None] < np.arange(P)[None, :]).astype(np.float32)
    u8 = (np.arange(E)[:, None] < np.arange(E)[None, :]).astype(np.float32)

    in_maps = []
    for c in range(NCORES):
        qrows = qkv_w[2 * c * HD:(2 * c + 2) * HD]
        krows = qkv_w[NH * HD + (c // 2) * HD: NH * HD + (c // 2 + 1) * HD]
        vrows = qkv_w[(NH + NKV) * HD + (c // 2) * HD:
                      (NH + NKV) * HD + (c // 2 + 1) * HD]
        qkv_sh = np.concatenate([qrows, krows, vrows], axis=0)   # [256, H]
        qkvwT = np.ascontiguousarray(qkv_sh.T)                   # [H, 256]

        wsT = ws[c].T                                            # [H, 2I]
        wspb = np.ascontiguousarray(
            wsT.reshape(KT, P, 2 * JT, P).transpose(2, 1, 0, 3)
               .reshape(2 * JT, P, KT * P)).astype(ml_dtypes.bfloat16)
        w2T = w2s[c].T                                           # [I, H]
        w2tp = np.ascontiguousarray(
            w2T.reshape(JT, P, H).transpose(1, 0, 2)).astype(ml_dtypes.bfloat16)

        htok = np.ascontiguousarray(hidden_states[c * P:(c + 1) * P, :])
        colcv = np.full((P, 1), float(c), np.float32)

        in_maps.append({
            "hT": hT, "htok": htok, "qkvwT": qkvwT, "owTf": owTf,
            "ln1w": ln1p, "ln2row": ln2row, "rwT": rwT,
            "wspb": wspb, "w2tp": w2tp, "amask": amask,
            "iota_row": iota_row, "ones128": ones128, "ones1r": ones1r,
            "ones2": ones2, "colc": colcv, "tokidx": tokidx,
            "l128": l128, "u8": u8,
        })
    return in_maps


def _host_reference(hidden_states, ln1_w, qkv_w, o_w, ln2_w, router_w, ws, w2s):
    """Numpy fallback (only used if an expert exceeds the 384-token capacity,
    which cannot happen for headroom-style inputs; kept for safety)."""
    x = np.asarray(hidden_states, np.float32)

    def rms(v, w):
        var = (v * v).mean(-1, keepdims=True)
        return v / np.sqrt(var + EPS) * w

    h = rms(x, ln1_w)
    qkv = h @ qkv_w.T
    q = qkv[:, :NH * HD].reshape(T, NH, HD)
    k = qkv[:, NH * HD:(NH + NKV) * HD].reshape(T, NKV, HD)
    v = qkv[:, (NH + NKV) * HD:].reshape(T, NKV, HD)
    rep = NH // NKV
    k = np.repeat(k, rep, axis=1)
    v = np.repeat(v, rep, axis=1)
    sc = np.einsum('qhd,khd->hqk', q, k) * SCALE
    ii = np.arange(T)[:, None]
    jj = np.arange(T)[None, :]
    mask = (jj <= ii) & ((ii - jj) < SW)
    sc = np.where(mask[None], sc, NEG)
    sc -= sc.max(-1, keepdims=True)
    p = np.exp(sc)
    p /= p.sum(-1, keepdims=True)
    attn = np.einsum('hqk,khd->qhd', p, v).reshape(T, NH * HD)
    resid = x + attn @ o_w.T
    h2 = rms(resid, ln2_w)
    logits = h2 @ router_w.T
    lm = logits.max(-1, keepdims=True)
    pe = np.exp(logits - lm)
    probs = pe / pe.sum(-1, keepdims=True)
    order = np.argsort(-probs, axis=1)[:, :2]
    moe = np.zeros((T, H), np.float32)
    for e in range(E):
        sel = (order == e).any(axis=1)
        wsel = np.where(sel, probs[:, e], 0.0)
        gu = h2 @ ws[e].T
        g, u = gu[:, :I], gu[:, I:]
        a = (g / (1.0 + np.exp(-g))) * u
        moe += wsel[:, None] * (a @ w2s[e].T)
    return moe, resid


def kernel(hidden_states, positions, ln1_w, qkv_w, o_w, ln2_w, router_w, ws, w2s):
    nc = _get_nc()
    in_maps = _pack_inputs(hidden_states, ln1_w, qkv_w, o_w, ln2_w,
                           router_w, ws, w2s)
    res = run_bass_kernel_spmd(nc, in_maps, list(range(NCORES)))
    counts = [float(res.results[c]["cnt"][0, 0]) for c in range(NCORES)]
    if max(counts) > C:
        return _host_reference(hidden_states, ln1_w, qkv_w, o_w, ln2_w,
                               router_w, ws, w2s)
    moe_out = np.concatenate(
        [np.asarray(res.results[c]["moe_sl"], np.float32)
         for c in range(NCORES)], axis=0)                        # [T, H]
    residual = np.concatenate(
        [np.asarray(res.results[c]["resid_sl"], np.float32)
         for c in range(NCORES)], axis=0)                        # [T, H]
    return moe_out, residual


# revision 47
# speedup vs baseline: 15848.4356x; 1.0978x over previous
"""Trainium2 Bass kernel for nn_JambaAttentionDecoderLayer (8-core SPMD).

v2: routed MoE + restructured collectives.

Sharding: tensor-parallel attention (2 q-heads + 1 kv-head per core) with an
AllToAll feature->token exchange, token-sliced o-proj/rmsnorm/router
(128 tokens per core), and expert parallelism for the MoE with true top-2
token routing: per-expert token index lists are built on device with
matmul-based stream compaction, tokens gathered with the DGE dma_gather
(transpose mode, bf16), expert FFN runs on a fixed 384-token capacity
(vs 1024 dense), and results are scattered back with a 0/1 scatter-matmul
followed by a bf16 ReduceScatter.

Precision: the attention -> residual -> rmsnorm -> router-logits path stays
in f32/f32r so the top-2 expert *selection* matches the f32 reference
(min |logit2-logit3| gap for this model is ~5e-4; bf16 noise would flip
experts). Everything downstream of selection (expert inputs, weights,
combine) runs in bf16: errors there scale with the element magnitude and
stay well inside the 2e-2 gate.
"""

import os as _os
import numpy as np

import concourse.bass as bass
import concourse.tile as tile
import concourse.mybir as mybir
from concourse import bacc
from concourse.bass_utils import run_bass_kernel_spmd

# dims (hardcoded per spec)
T = 1024
H = 1024
NH = 16
NKV = 4
HD = 64
I = 2816
E = 8
SW = 512
EPS = 1e-6
SCALE = HD ** -0.5

NCORES = 8
P = 128
KT = H // P          # 8 k-tiles over H
JT = I // P          # 22 j-tiles over I
C = 384              # MoE token capacity per expert (max count here is 275)
RC = C // P          # 3 slot chunks
NEG = -1.0e30

f32 = mybir.dt.float32
f32r = mybir.dt.float32r
bf16 = mybir.dt.bfloat16
i16 = mybir.dt.int16

# attention mask offsets: off = q_tile_start - k_tile_start for [128k,512q] tiles
OFFS = [-384, -256, -128, 0, 128, 256, 384, 512]
QT_KIS = {0: list(range(0, 4)), 1: list(range(0, 8))}

AxX = mybir.AxisListType.X
Alu = mybir.AluOpType
Act = mybir.ActivationFunctionType


def _build(profile=False):
    ndev = 1 if profile else NCORES
    nc = bacc.Bacc("TRN2", target_bir_lowering=False, debug=False,
                   num_devices=ndev)

    # ---- kernel I/O ----
    hT_d = nc.dram_tensor("hT", [H, T], f32, kind="ExternalInput")
    htok_d = nc.dram_tensor("htok", [P, H], f32, kind="ExternalInput")
    qkvwT_d = nc.dram_tensor("qkvwT", [H, 256], f32r, kind="ExternalInput")
    owTf_d = nc.dram_tensor("owTf", [P, KT, H], f32r, kind="ExternalInput")
    ln1w_d = nc.dram_tensor("ln1w", [P, KT], f32, kind="ExternalInput")
    ln2row_d = nc.dram_tensor("ln2row", [1, H], f32r, kind="ExternalInput")
    rwT_d = nc.dram_tensor("rwT", [P, KT, E], f32, kind="ExternalInput")
    wspb_d = nc.dram_tensor("wspb", [2 * JT, P, KT * P], bf16,
                            kind="ExternalInput")
    w2tp_d = nc.dram_tensor("w2tp", [P, JT, H], bf16, kind="ExternalInput")
    amask_d = nc.dram_tensor("amask", [len(OFFS), P, 512], bf16,
                             kind="ExternalInput")
    iota_row_d = nc.dram_tensor("iota_row", [1, T], f32r, kind="ExternalInput")
    ones128_d = nc.dram_tensor("ones128", [P, 1], f32r, kind="ExternalInput")
    ones1r_d = nc.dram_tensor("ones1r", [1, P], f32r, kind="ExternalInput")
    colc_d = nc.dram_tensor("colc", [P, 1], f32, kind="ExternalInput")
    tokidx_d = nc.dram_tensor("tokidx", [P, KT], f32r, kind="ExternalInput")
    l128_d = nc.dram_tensor("l128", [P, P], f32r, kind="ExternalInput")
    u8_d = nc.dram_tensor("u8", [E, E], f32r, kind="ExternalInput")
    ones2_d = nc.dram_tensor("ones2", [P, 2], f32r, kind="ExternalInput")

    resid_sl_d = nc.dram_tensor("resid_sl", [P, H], f32, kind="ExternalOutput")
    moe_sl_d = nc.dram_tensor("moe_sl", [P, H], bf16, kind="ExternalOutput")
    cnt_d = nc.dram_tensor("cnt", [1, 1], f32, kind="ExternalOutput")

    rg = [list(range(NCORES))]

    import contextlib
    lp = getattr(nc, "allow_low_precision", None)
    lp_cm = lp(reason="f32r/bf16 matmul operands; within rel-err budget") \
        if lp else contextlib.nullcontext()
    with lp_cm, tile.TileContext(nc) as tc:
        with tc.tile_pool(name="const", bufs=1) as cpool, \
             tc.tile_pool(name="persist", bufs=1) as pers, \
             tc.tile_pool(name="dram", bufs=1, space="DRAM") as dram:

            # ---- constants ----
            ones128 = cpool.tile([P, 1], f32r)
            nc.sync.dma_start(ones128[:], ones128_d[:])
            ones1r = cpool.tile([1, P], f32r)
            nc.sync.dma_start(ones1r[:], ones1r_d[:])
            ln1w = cpool.tile([P, KT], f32)
            nc.sync.dma_start(ln1w[:], ln1w_d[:])
            colc = cpool.tile([P, 1], f32)
            nc.sync.dma_start(colc[:], colc_d[:])
            tokidx = cpool.tile([P, KT], f32r)
            nc.sync.dma_start(tokidx[:], tokidx_d[:])
            l128 = cpool.tile([P, P], f32r)
            nc.sync.dma_start(l128[:], l128_d[:])
            u8 = cpool.tile([E, E], f32r)
            nc.sync.dma_start(u8[:], u8_d[:])
            ones2 = cpool.tile([P, 2], f32r)
            nc.sync.dma_start(ones2[:], ones2_d[:])
            ident = cpool.tile([P, P], f32)
            from concourse.masks import make_identity
            make_identity(nc, ident[:])
            iota_row = cpool.tile([1, T], f32r)
            nc.sync.dma_start(iota_row[:], iota_row_d[:])

            htok = pers.tile([P, H], f32)
            nc.sync.dma_start(htok[:], htok_d[:])
            rwT = pers.tile([P, KT, E], f32)
            nc.sync.dma_start(rwT[:], rwT_d[:])
            ln2row = pers.tile([1, H], f32r)
            nc.sync.dma_start(ln2row[:], ln2row_d[:])

            # broadcast rows: iota [128, T] and ln2w [128, H]
            iotaT = pers.tile([P, T], f32)
            ln2bc = pers.tile([P, H], f32)
            with tc.tile_pool(name="bc", bufs=1, space="PSUM") as bcps:
                for ni in range(2):
                    pbi = bcps.tile([P, 512], f32, tag="pbi", bufs=2)
                    nc.tensor.matmul(pbi[:], ones1r[:],
                                     iota_row[:, ni * 512:(ni + 1) * 512],
                                     start=True, stop=True)
                    nc.vector.tensor_copy(iotaT[:, ni * 512:(ni + 1) * 512],
                                          pbi[:])
                    pbl = bcps.tile([P, 512], f32, tag="pbl", bufs=2)
                    nc.tensor.matmul(pbl[:], ones1r[:],
                                     ln2row[:, ni * 512:(ni + 1) * 512],
                                     start=True, stop=True)
                    nc.vector.tensor_copy(ln2bc[:, ni * 512:(ni + 1) * 512],
                                          pbl[:])

            # dram bounce buffers for collectives
            a2a_in = dram.tile([KT, P, P], f32)
            a2a_out = dram.tile([KT, P, P], f32)
            ag2a_in = dram.tile([P, H], bf16)
            ag2a_out = dram.tile([T, H], bf16, addr_space="Shared")
            ag2b_in = dram.tile([P, 32], f32)
            ag2b_out = dram.tile([T, 32], f32, addr_space="Shared")
            idx_d = dram.tile([1, C], i16)
            rs_in = dram.tile([T, H], bf16)
            rs_out = dram.tile([P, H], bf16)

            # =========== feature-major RMSNorm (ln1), as baseline ===========
            def rmsnorm_fm(src_tile, lnw_tile, dst_tile):
                with tc.tile_pool(name="rn", bufs=1) as tmp, \
                     tc.tile_pool(name="rnps", bufs=1, space="PSUM") as psum:
                    vs = [None, None]
                    for ni in range(2):
                        pv = psum.tile([1, 512], f32, tag="pvar")
                        for k in range(KT):
                            sq = tmp.tile([P, 512], f32r, tag="sq", bufs=2)
                            nc.scalar.activation(
                                sq[:], src_tile[:, k, ni * 512:(ni + 1) * 512],
                                Act.Square)
                            nc.tensor.matmul(pv[:], ones128[:], sq[:],
                                             start=(k == 0), stop=(k == KT - 1))
                        v = tmp.tile([1, 512], f32, tag="vv")
                        nc.vector.tensor_scalar(v[:], pv[:], 1.0 / H, EPS,
                                                Alu.mult, Alu.add)
                        sd = tmp.tile([1, 512], f32, tag="sd")
                        nc.scalar.activation(sd[:], v[:], Act.Sqrt)
                        s = tmp.tile([1, 512], f32r, tag="ss")
                        nc.vector.reciprocal(s[:], sd[:])
                        pb = psum.tile([P, 512], f32, tag="pbc", bufs=2)
                        nc.tensor.matmul(pb[:], ones1r[:], s[:],
                                         start=True, stop=True)
                        vs[ni] = pb
                    for ni in range(2):
                        for k in range(KT):
                            nc.vector.scalar_tensor_tensor(
                                dst_tile[:, k, ni * 512:(ni + 1) * 512],
                                src_tile[:, k, ni * 512:(ni + 1) * 512],
                                lnw_tile[:, k:k + 1],
                                vs[ni][:],
                                Alu.mult, Alu.mult)

            # =========== phase 1: ln1 + qkv + attention (f32r, as baseline) ==
            with tc.tile_pool(name="hp", bufs=1) as hp:
                hT = hp.tile([P, KT, T], f32)
                nc.sync.dma_start(hT[:], hT_d.rearrange("(k p) t -> p k t", p=P))

                with tc.tile_pool(name="p1", bufs=2) as p1:
                    qkvT = p1.tile([P, 2, T], f32r, bufs=1)
                    with tc.tile_pool(name="p1a", bufs=1) as p1a:
                        hnT = p1a.tile([P, KT, T], f32r)
                        rmsnorm_fm(hT, ln1w, hnT)

                        with tc.tile_pool(name="ps1", bufs=1, space="PSUM") as ps1:
                            qkvw = p1a.tile([P, KT, 256], f32r)
                            nc.sync.dma_start(
                                qkvw[:], qkvwT_d.rearrange("(k p) m -> p k m", p=P))
                            for mi in range(2):
                                for ni in range(2):
                                    pq = ps1.tile([P, 512], f32, tag="pqkv", bufs=2)
                                    for k in range(KT):
                                        nc.tensor.matmul(
                                            pq[:], qkvw[:, k, mi * P:(mi + 1) * P],
                                            hnT[:, k, ni * 512:(ni + 1) * 512],
                                            start=(k == 0), stop=(k == KT - 1))
                                    nc.vector.tensor_copy(
                                        qkvT[:, mi, ni * 512:(ni + 1) * 512], pq[:])

                    # v to token-major [128tok, 8tiles, 64]
                    v_sb = p1.tile([P, KT, HD], f32r, bufs=1)
                    with tc.tile_pool(name="ps1v", bufs=1, space="PSUM") as ps1v:
                        for ti in range(KT):
                            pvt = ps1v.tile([P, HD], f32, tag="pvt", bufs=2)
                            nc.tensor.transpose(
                                pvt[:],
                                qkvT[HD:P, 1, ti * P:(ti + 1) * P].bitcast(f32),
                                ident[HD:P, HD:P])
                            nc.vector.tensor_copy(v_sb[:, ti, :], pvt[:])

                    attn_sb = p1.tile([HD, 2, T], f32r, bufs=1)
                    am = p1.tile([P, len(OFFS), 512], bf16, bufs=1)
                    nc.sync.dma_start(am[:], amask_d.rearrange("o p f -> p o f"))

                    # re-base head-1 q to partitions 0..63 (SBUF->SBUF DMA)
                    q1_sb = p1.tile([HD, T], f32r, bufs=1)
                    nc.sync.dma_start(q1_sb[:], qkvT[HD:P, 0, :])

                    with tc.tile_pool(name="ps1b", bufs=1, space="PSUM") as ps1b:
                        for h in range(2):
                            qT = qkvT[0:HD, 0, :] if h == 0 else q1_sb[:]
                            kTT = qkvT[0:HD, 1, :]
                            for qt in range(2):
                                kis = QT_KIS[qt]
                                ppv = ps1b.tile([HD, 512], f32, tag="ppv")
                                pcs = ps1b.tile([1, 512], f32, tag="pcs")
                                for idx, ki in enumerate(kis):
                                    pscore = ps1b.tile([P, 512], f32,
                                                       tag="pscore", bufs=2)
                                    nc.tensor.matmul(
                                        pscore[:], kTT[:, ki * P:(ki + 1) * P],
                                        qT[:, qt * 512:(qt + 1) * 512],
                                        start=True, stop=True)
                                    off_i = OFFS.index(qt * 512 - ki * P)
                                    sm = p1.tile([P, 512], f32, tag="sm")
                                    nc.vector.scalar_tensor_tensor(
                                        sm[:], pscore[:], SCALE,
                                        am[:, off_i, :], Alu.mult, Alu.add)
                                    pexp = p1.tile([P, 512], f32r, tag="pexp")
                                    nc.scalar.activation(pexp[:], sm[:], Act.Exp)
                                    nc.tensor.matmul(
                                        pcs[:], ones128[:], pexp[:],
                                        start=(idx == 0),
                                        stop=(idx == len(kis) - 1))
                                    nc.tensor.matmul(
                                        ppv[:], v_sb[:, ki, :], pexp[:],
                                        start=(idx == 0),
                                        stop=(idx == len(kis) - 1))
                                inv = p1.tile([1, 512], f32r, tag="inv")
                                nc.vector.reciprocal(inv[:], pcs[:])
                                pbc = ps1b.tile([P, 512], f32, tag="pbc2")
                                nc.tensor.matmul(pbc[:], ones1r[:], inv[:],
                                                 start=True, stop=True)
                                binv = p1.tile([HD, 512], f32, tag="binv")
                                nc.vector.tensor_copy(binv[:], pbc[:HD, :])
                                nc.vector.tensor_tensor(
                                    attn_sb[:, h, qt * 512:(qt + 1) * 512],
                                    ppv[:], binv[:], Alu.mult)

                    # feature->token AllToAll: block b = my 128 attn features
                    # for token tile b
                    for b in range(KT):
                        nc.sync.dma_start(
                            a2a_in[b, :, :].rearrange("(h d) t -> d h t", h=2),
                            attn_sb[:, :, b * P:(b + 1) * P].bitcast(f32))
                    if not profile and not _os.environ.get("NOA2A"):
                        nc.gpsimd.collective_compute(
                            "AllToAll", Alu.bypass, replica_groups=rg,
                            ins=[a2a_in[:]], outs=[a2a_out[:]])
                    elif not profile:
                        nc.sync.dma_start(a2a_out[:], a2a_in[:])
            # hT pool closed here

            # =========== phase 2: token-sliced o-proj + resid + ln2 + router =
            w2pool_cm = tc.tile_pool(name="w2pool", bufs=1)
            w2pool = w2pool_cm.__enter__()
            w2tp = w2pool.tile([P, JT, H], bf16)
            nc.sync.dma_start(w2tp[:], w2tp_d[:])

            h2f = pers.tile([P, H], f32)
            wtm_pack = pers.tile([P, 32], f32)
            with tc.tile_pool(name="p2", bufs=1) as p2, \
                 tc.tile_pool(name="ps2", bufs=1, space="PSUM") as ps2:
                owT = p2.tile([P, KT, H], f32r)
                nc.sync.dma_start(owT[:], owTf_d[:])
                af = p2.tile([P, KT, P], f32r)
                src = a2a_in if profile else a2a_out
                nc.sync.dma_start(
                    af[:], src[:].rearrange("k f t -> f k t").bitcast(f32r))

                resid = p2.tile([P, H], f32)
                for ni in range(2):
                    po = ps2.tile([P, 512], f32, tag="po", bufs=2)
                    for k in range(KT):
                        nc.tensor.matmul(po[:], af[:, k, :],
                                         owT[:, k, ni * 512:(ni + 1) * 512],
                                         start=(k == 0), stop=(k == KT - 1))
                    nc.vector.tensor_tensor(resid[:, ni * 512:(ni + 1) * 512],
                                            po[:],
                                            htok[:, ni * 512:(ni + 1) * 512],
                                            Alu.add)
                nc.sync.dma_start(resid_sl_d[:], resid[:])

                # token-major rmsnorm (ln2) for this 128-token slice
                sq2 = p2.tile([P, H], f32)
                nc.scalar.activation(sq2[:], resid[:], Act.Square)
                var = p2.tile([P, 1], f32)
                nc.vector.reduce_sum(var[:], sq2[:], axis=AxX)
                v2 = p2.tile([P, 1], f32)
                nc.vector.tensor_scalar(v2[:], var[:], 1.0 / H, EPS,
                                        Alu.mult, Alu.add)
                sd2 = p2.tile([P, 1], f32)
                nc.scalar.activation(sd2[:], v2[:], Act.Sqrt)
                inv2 = p2.tile([P, 1], f32)
                nc.vector.reciprocal(inv2[:], sd2[:])
                nc.vector.scalar_tensor_tensor(h2f[:], resid[:], inv2[:],
                                               ln2bc[:], Alu.mult, Alu.mult)

                # router on own 128 tokens: transpose h2 -> feature-major
                h2T_sl = p2.tile([P, KT, P], f32)
                for k in range(KT):
                    ptk = ps2.tile([P, P], f32, tag="ptk", bufs=2)
                    nc.tensor.transpose(
                        ptk[:], h2f[:, k * P:(k + 1) * P], ident[:])
                    nc.vector.tensor_copy(h2T_sl[:, k, :], ptk[:])

                plog = ps2.tile([E, P], f32, tag="plog")
                for k in range(KT):
                    nc.tensor.matmul(plog[:], rwT[:, k, :], h2T_sl[:, k, :],
                                     start=(k == 0), stop=(k == KT - 1))
                logsb = p2.tile([E, P], f32)
                nc.vector.tensor_copy(logsb[:], plog[:])
                ptr = ps2.tile([P, E], f32, tag="ptr")
                nc.tensor.transpose(ptr[:], logsb[:], ident[:E, :E])
                lg = p2.tile([P, E], f32)
                nc.vector.tensor_copy(lg[:], ptr[:])

                # softmax + top-2 (f32, selection-exact)
                m1 = p2.tile([P, 1], f32)
                nc.vector.reduce_max(m1[:], lg[:], axis=AxX)
                nm1 = p2.tile([P, 1], f32)
                nc.vector.tensor_scalar_mul(nm1[:], m1[:], -1.0)
                ex = p2.tile([P, E], f32)
                nc.scalar.activation(ex[:], lg[:], Act.Exp, bias=nm1[:])
                den = p2.tile([P, 1], f32)
                nc.vector.reduce_sum(den[:], ex[:], axis=AxX)
                dinv = p2.tile([P, 1], f32)
                nc.vector.reciprocal(dinv[:], den[:])
                probs = p2.tile([P, E], f32)
                nc.vector.tensor_scalar_mul(probs[:], ex[:], dinv[:])

                w1 = p2.tile([P, 1], f32)
                nc.vector.reduce_max(w1[:], probs[:], axis=AxX)
                eq1 = p2.tile([P, E], f32)
                nc.vector.tensor_scalar(eq1[:], probs[:], w1[:], None,
                                        Alu.is_ge)
                it1 = p2.tile([P, E], f32)
                nc.vector.tensor_tensor(it1[:], eq1[:], iotaT[:, :E], Alu.mult)
                i1 = p2.tile([P, 1], f32)
                nc.vector.reduce_max(i1[:], it1[:], axis=AxX)
                pm = p2.tile([P, E], f32)
                nc.vector.tensor_tensor(pm[:], probs[:], eq1[:], Alu.mult)
                masked = p2.tile([P, E], f32)
                nc.vector.tensor_tensor(masked[:], probs[:], pm[:],
                                        Alu.subtract)
                w2 = p2.tile([P, 1], f32)
                nc.vector.reduce_max(w2[:], masked[:], axis=AxX)
                eq2 = p2.tile([P, E], f32)
                nc.vector.tensor_scalar(eq2[:], masked[:], w2[:], None,
                                        Alu.is_ge)
                it2 = p2.tile([P, E], f32)
                nc.vector.tensor_tensor(it2[:], eq2[:], iotaT[:, :E], Alu.mult)
                i2 = p2.tile([P, 1], f32)
                nc.vector.reduce_max(i2[:], it2[:], axis=AxX)

                nc.vector.memset(wtm_pack[:, 4:32], 0.0)
                nc.vector.tensor_copy(wtm_pack[:, 0:1], w1[:])
                nc.vector.tensor_copy(wtm_pack[:, 1:2], w2[:])
                nc.vector.tensor_copy(wtm_pack[:, 2:3], i1[:])
                nc.vector.tensor_copy(wtm_pack[:, 3:4], i2[:])
                nc.sync.dma_start(ag2b_in[:], wtm_pack[:])

                # h2 (bf16) to DRAM for the expert gather -- staged after the
                # router pack so the tiny AG2b wins the CC queue and unblocks
                # the index build while the big AG2a is still in flight
                h2bf = p2.tile([P, H], bf16)
                nc.vector.tensor_copy(h2bf[:], h2f[:])
                nc.sync.dma_start(ag2a_in[:], h2bf[:])

                if not profile and not _os.environ.get("NOAGB"):
                    nc.gpsimd.collective_compute(
                        "AllGather", Alu.bypass, replica_groups=rg,
                        ins=[ag2b_in[:]], outs=[ag2b_out[:]])
                elif not profile:
                    nc.sync.dma_start(ag2b_out[0:P, :], ag2b_in[:])
                if not profile and not _os.environ.get("NOAGA"):
                    nc.gpsimd.collective_compute(
                        "AllGather", Alu.bypass, replica_groups=rg,
                        ins=[ag2a_in[:]], outs=[ag2a_out[:]])
                elif not profile:
                    nc.sync.dma_start(ag2a_out[0:P, :], ag2a_in[:])

            # =========== phase 3: routing index build for my expert ==========
            moepool_cm = tc.tile_pool(name="moepool", bufs=1)
            moepool = moepool_cm.__enter__()
            gat = moepool.tile([P, RC], f32)     # per-slot combine weight
            S0 = moepool.tile([P, RC, T], bf16)  # per-slot 0/1 scatter rows
            idxs_sb = moepool.tile([P, C // 16], i16)
            h2c = moepool.tile([P, KT, C], bf16)

            with tc.tile_pool(name="p3", bufs=1) as p3, \
                 tc.tile_pool(name="ps3", bufs=1, space="PSUM") as ps3:
                wtm_src = ag2b_in if profile else ag2b_out
                wtm_sb = p3.tile([P, KT, 4], f32)
                nc.sync.dma_start(
                    wtm_sb[:],
                    wtm_src[:, 0:4].rearrange("(ti p) k -> p ti k", p=P))

                # wcol[t] = weight of my expert for token t (0 if unselected)
                eqa = p3.tile([P, KT], f32)
                nc.vector.tensor_scalar(eqa[:], wtm_sb[:, :, 2], colc[:], None,
                                        Alu.is_equal)
                wa = p3.tile([P, KT], f32)
                nc.vector.tensor_tensor(wa[:], eqa[:], wtm_sb[:, :, 0],
                                        Alu.mult)
                eqb = p3.tile([P, KT], f32)
                nc.vector.tensor_scalar(eqb[:], wtm_sb[:, :, 3], colc[:], None,
                                        Alu.is_equal)
                wb_ = p3.tile([P, KT], f32)
                nc.vector.tensor_tensor(wb_[:], eqb[:], wtm_sb[:, :, 1],
                                        Alu.mult)
                wcol = p3.tile([P, KT], f32r)
                nc.vector.tensor_tensor(wcol[:], wa[:], wb_[:], Alu.add)
                ind = p3.tile([P, KT], f32)
                nc.vector.tensor_scalar(ind[:], wcol[:], 0.0, None, Alu.is_gt)
                ind_r = p3.tile([P, KT], f32r)
                nc.vector.tensor_copy(ind_r[:], ind[:])

                # counts per tile -> exclusive offsets (row form)
                pcnt = ps3.tile([KT, 2], f32, tag="pcnt")
                nc.tensor.matmul(pcnt[:], ind_r[:], ones2[:],
                                 start=True, stop=True)
                cnts = p3.tile([KT, 1], f32r)
                nc.vector.tensor_copy(cnts[:], pcnt[:, 0:1])
                # total count (for host-side capacity check)
                ptot = ps3.tile([1, 2], f32, tag="ptot")
                nc.tensor.matmul(ptot[:], cnts[:], ones2[0:KT, :],
                                 start=True, stop=True)
                ctot = p3.tile([1, 1], f32)
                nc.vector.tensor_copy(ctot[:], ptot[:, 0:1])
                nc.sync.dma_start(cnt_d[:], ctot[:])

                poff = ps3.tile([1, KT], f32, tag="poff")
                nc.tensor.matmul(poff[:], cnts[:], u8[:],
                                 start=True, stop=True)
                offsrow = p3.tile([1, KT], f32r)
                nc.vector.tensor_copy(offsrow[:], poff[:])

                # global rank of each token within my expert's list
                prank = ps3.tile([P, KT], f32, tag="prank")
                nc.tensor.matmul(prank[:], l128[:], ind_r[:],
                                 start=True, stop=False)
                nc.tensor.matmul(prank[:], ones1r[:], offsrow[:],
                                 start=False, stop=True)
                grank = p3.tile([P, KT], f32)
                nc.vector.tensor_copy(grank[:], prank[:])

                # M matrices + pos list + per-slot weights
                wcol2 = p3.tile([P, KT, 2], f32r)
                nc.vector.tensor_copy(wcol2[:, :, 0], wcol[:])
                nc.vector.tensor_copy(wcol2[:, :, 1], wcol[:])
                M8 = p3.tile([P, KT, C], f32r)
                for ti in range(KT):
                    nc.vector.tensor_scalar(M8[:, ti, :], iotaT[:, :C],
                                            grank[:, ti:ti + 1],
                                            ind[:, ti:ti + 1],
                                            Alu.is_equal, Alu.mult)
                ppos = ps3.tile([1, C], f32, tag="ppos")
                for ti in range(KT):
                    nc.tensor.matmul(ppos[:], tokidx[:, ti:ti + 1],
                                     M8[:, ti, :],
                                     start=(ti == 0), stop=(ti == KT - 1))
                pwsl = ps3.tile([P, RC, 2], f32, tag="pwsl")
                for ch in range(RC):
                    for ti in range(KT):
                        nc.tensor.matmul(pwsl[:, ch, :],
                                         M8[:, ti, ch * P:(ch + 1) * P],
                                         wcol2[:, ti, :],
                                         start=(ti == 0), stop=(ti == KT - 1))
                nc.vector.tensor_copy(gat[:], pwsl[:, :, 0])

                pos_sb = p3.tile([1, C], f32r)
                nc.vector.tensor_copy(pos_sb[:], ppos[:])
                pos_i16 = p3.tile([1, C], i16)
                nc.vector.tensor_copy(pos_i16[:], pos_sb[:])
                nc.sync.dma_start(idx_d[:], pos_i16[:])
                # wrap to [16, C/16] and replicate to all 128 partitions
                nc.sync.dma_start(idxs_sb[0:16, :],
                                  idx_d[0, :].rearrange("(s p) -> p s", p=16))
                nc.sync.dma_start(idxs_sb[16:32, :], idxs_sb[0:16, :])
                nc.sync.dma_start(idxs_sb[32:64, :], idxs_sb[0:32, :])
                nc.sync.dma_start(idxs_sb[64:128, :], idxs_sb[0:64, :])

                # gather my expert's tokens (bf16, feature-major)
                h2src = ag2a_in if profile else ag2a_out
                nc.gpsimd.dma_gather(
                    h2c[:], h2src[:], idxs_sb[:], C, C, H,
                    transpose=True)

                # slot position columns -> S0 scatter rows
                posch = p3.tile([P, RC], f32)
                for ch in range(RC):
                    ptp = ps3.tile([P, 1], f32, tag="ptp", bufs=2)
                    nc.tensor.transpose(
                        ptp[:], pos_sb[0:1, ch * P:(ch + 1) * P].bitcast(f32),
                        ident[0:1, 0:1])
                    nc.vector.tensor_copy(posch[:, ch:ch + 1], ptp[:])
                for ch in range(RC):
                    nc.vector.tensor_scalar(S0[:, ch, :], iotaT[:],
                                            posch[:, ch:ch + 1], None,
                                            Alu.is_equal)

            # =========== phase 4: expert FFN on C tokens =====================
            with tc.tile_pool(name="wpool", bufs=2) as wpool, \
                 tc.tile_pool(name="apool", bufs=1) as apool, \
                 tc.tile_pool(name="spool", bufs=2) as spool, \
                 tc.tile_pool(name="ps4", bufs=1, space="PSUM") as ps4:
                act = apool.tile([P, JT, C], bf16)
                for j in range(JT):
                    wg = wpool.tile([P, KT * P], bf16, tag="wg", bufs=2)
                    nc.sync.dma_start(wg[:], wspb_d[j])
                    wu = wpool.tile([P, KT * P], bf16, tag="wu", bufs=2)
                    nc.sync.dma_start(wu[:], wspb_d[JT + j])
                    pg = ps4.tile([P, C], f32, tag="pg", bufs=2)
                    pu = ps4.tile([P, C], f32, tag="pu", bufs=2)
                    for k in range(KT):
                        nc.tensor.matmul(pg[:], wg[:, k * P:(k + 1) * P],
                                         h2c[:, k, :],
                                         start=(k == 0), stop=(k == KT - 1))
                    for k in range(KT):
                        nc.tensor.matmul(pu[:], wu[:, k * P:(k + 1) * P],
                                         h2c[:, k, :],
                                         start=(k == 0), stop=(k == KT - 1))
                    sil = spool.tile([P, C], f32, tag="sil")
                    nc.scalar.activation(sil[:], pg[:], Act.Silu)
                    nc.vector.tensor_tensor(act[:, j, :], sil[:], pu[:],
                                            Alu.mult)

                # down proj, slot-major output [slots, H], scaled by gatings
                cmp_bf = apool.tile([P, RC, H], bf16)
                for ch in range(RC):
                    for ni in range(2):
                        pd = ps4.tile([P, 512], f32, tag="pd", bufs=2)
                        for j in range(JT):
                            nc.tensor.matmul(
                                pd[:], act[:, j, ch * P:(ch + 1) * P],
                                w2tp[:, j, ni * 512:(ni + 1) * 512],
                                start=(j == 0), stop=(j == JT - 1))
                        nc.vector.tensor_scalar(
                            cmp_bf[:, ch, ni * 512:(ni + 1) * 512], pd[:],
                            gat[:, ch:ch + 1], None, Alu.mult)

                # scatter back to dense [T, H] via 0/1 scatter-matmul
                for tt in range(KT):
                    for ni in range(2):
                        psc = ps4.tile([P, 512], f32, tag="psc", bufs=2)
                        for ch in range(RC):
                            nc.tensor.matmul(
                                psc[:], S0[:, ch, tt * P:(tt + 1) * P],
                                cmp_bf[:, ch, ni * 512:(ni + 1) * 512],
                                start=(ch == 0), stop=(ch == RC - 1))
                        eo = spool.tile([P, 512], bf16, tag="eo")
                        nc.vector.tensor_copy(eo[:], psc[:])
                        nc.sync.dma_start(
                            rs_in[tt * P:(tt + 1) * P,
                                  ni * 512:(ni + 1) * 512], eo[:])

                if not profile and not _os.environ.get("NORS"):
                    nc.gpsimd.collective_compute(
                        "ReduceScatter", Alu.add, replica_groups=rg,
                        ins=[rs_in[:]], outs=[rs_out[:]])
                elif not profile:
                    nc.sync.dma_start(rs_out[:], rs_in[0:P, :])
                out_sb = spool.tile([P, H], bf16, tag="osb")
                nc.sync.dma_start(out_sb[:],
                                  rs_out[:] if not profile else rs_in[0:P, :])
                nc.sync.dma_start(moe_sl_d[:], out_sb[:])

            moepool_cm.__exit__(None, None, None)
            w2pool_cm.__exit__(None, None, None)

    nc.compile()
    return nc


_NC = None


def _get_nc():
    global _NC
    if _NC is None:
        _NC = _build()
    return _NC


def _pack_inputs(hidden_states, ln1_w, qkv_w, o_w, ln2_w, router_w, ws, w2s):
    import ml_dtypes
    hidden_states = np.asarray(hidden_states, np.float32)
    qkv_w = np.asarray(qkv_w, np.float32)
    o_w = np.asarray(o_w, np.float32)
    router_w = np.asarray(router_w, np.float32)
    ws = np.asarray(ws, np.float32)
    w2s = np.asarray(w2s, np.float32)
    ln1_w = np.asarray(ln1_w, np.float32)
    ln2_w = np.asarray(ln2_w, np.float32)

    hT = np.ascontiguousarray(hidden_states.T)
    ln1p = np.ascontiguousarray(ln1_w.reshape(KT, P).T)
    ln2row = np.ascontiguousarray(ln2_w.reshape(1, H))
    rwT = np.ascontiguousarray(router_w.T.reshape(KT, P, E).transpose(1, 0, 2))
    owTf = np.ascontiguousarray(
        o_w.T.reshape(KT, P, H).transpose(1, 0, 2))

    amask = np.empty((len(OFFS), P, 512), np.float32)
    pp = np.arange(P)[:, None]
    ff = np.arange(512)[None, :]
    for i, off in enumerate(OFFS):
        d = off + ff - pp
        amask[i] = np.where((d >= 0) & (d < SW), 0.0, NEG)
    amask = amask.astype(ml_dtypes.bfloat16)

    ones128 = np.ones((P, 1), np.float32)
    ones1r = np.ones((1, P), np.float32)
    ones2 = np.ones((P, 2), np.float32)
    iota_row = np.arange(T, dtype=np.float32).reshape(1, T)
    tokidx = (np.arange(P)[:, None] + 128 * np.arange(KT)[None, :]) \
        .astype(np.float32)
    l128 = (np.arange(P)[:, None] < np.arange(P)[None, :]).astype(np.float32)
    u8 = (np.arange(E)[:, None] < np.arange(E)[None, :]).astype(np.float32)

    in_maps = []
    for c in range(NCORES):
        qrows = qkv_w[2 * c * HD:(2 * c + 2) * HD]
        krows = qkv_w[NH * HD + (c // 2) * HD: NH * HD + (c // 2 + 1) * HD]
        vrows = qkv_w[(NH + NKV) * HD + (c // 2) * HD:
                      (NH + NKV) * HD + (c // 2 + 1) * HD]
        qkv_sh = np.concatenate([qrows, krows, vrows], axis=0)   # [256, H]
        qkvwT = np.ascontiguousarray(qkv_sh.T)                   # [H, 256]

        wsT = ws[c].T                                            # [H, 2I]
        wspb = np.ascontiguousarray(
            wsT.reshape(KT, P, 2 * JT, P).transpose(2, 1, 0, 3)
               .reshape(2 * JT, P, KT * P)).astype(ml_dtypes.bfloat16)
        w2T = w2s[c].T                                           # [I, H]
        w2tp = np.ascontiguousarray(
            w2T.reshape(JT, P, H).transpose(1, 0, 2)).astype(ml_dtypes.bfloat16)

        htok = np.ascontiguousarray(hidden_states[c * P:(c + 1) * P, :])
        colcv = np.full((P, 1), float(c), np.float32)

        in_maps.append({
            "hT": hT, "htok": htok, "qkvwT": qkvwT, "owTf": owTf,
            "ln1w": ln1p, "ln2row": ln2row, "rwT": rwT,
            "wspb": wspb, "w2tp": w2tp, "amask": amask,
            "iota_row": iota_row, "ones128": ones128, "ones1r": ones1r,
            "ones2": ones2, "colc": colcv, "tokidx": tokidx,
            "l128": l128, "u8": u8,
        })
    return in_maps


def _host_reference(hidden_states, ln1_w, qkv_w, o_w, ln2_w, router_w, ws, w2s):
    """Numpy fallback (only used if an expert exceeds the 384-token capacity,
    which cannot happen for headroom-style inputs; kept for safety)."""
    x = np.asarray(hidden_states, np.float32)

    def rms(v, w):
        var = (v * v).mean(-1, keepdims=True)
        return v / np.sqrt(var + EPS) * w

    h = rms(x, ln1_w)
    qkv = h @ qkv_w.T
    q = qkv[:, :NH * HD].reshape(T, NH, HD)
    k = qkv[:, NH * HD:(NH + NKV) * HD].reshape(T, NKV, HD)
    v = qkv[:, (NH + NKV) * HD:].reshape(T, NKV, HD)
    rep = NH // NKV
    k = np.repeat(k, rep, axis=1)
    v = np.repeat(v, rep, axis=1)
    sc = np.einsum('qhd,khd->hqk', q, k) * SCALE
    ii = np.arange(T)[:, None]
    jj = np.arange(T)[None, :]
    mask = (jj <= ii) & ((ii - jj) < SW)
    sc = np.where(mask[None], sc, NEG)
    sc -= sc.max(-1, keepdims=True)
    p = np.exp(sc)
    p /= p.sum(-1, keepdims=True)
    attn = np.einsum('hqk,khd->qhd', p, v).reshape(T, NH * HD)
    resid = x + attn @ o_w.T
    h2 = rms(resid, ln2_w)
    logits = h2 @ router_w.T
    lm = logits.max(-1, keepdims=True)
    pe = np.exp(logits - lm)
    probs = pe / pe.sum(-1, keepdims=True)
    order = np.argsort(-probs, axis=1)[:, :2]
    moe = np.zeros((T, H), np.float32)
    for e in range(E):
        sel = (order == e).any(axis=1)
        wsel = np.where(sel, probs[:, e], 0.0)
        gu = h2 @ ws[e].T
        g, u = gu[:, :I], gu[:, I:]
        a = (g / (1.0 + np.exp(-g))) * u
        moe += wsel[:, None] * (a @ w2s[e].T)
    return moe, resid


def kernel(hidden_states, positions, ln1_w, qkv_w, o_w, ln2_w, router_w, ws, w2s):
    nc = _get_nc()
    in_maps = _pack_inputs(hidden_states, ln1_w, qkv_w, o_w, ln2_w,
                           router_w, ws, w2s)
    res = run_bass_kernel_spmd(nc, in_maps, list(range(NCORES)))
    counts = [float(res.results[c]["cnt"][0, 0]) for c in range(NCORES)]
    if max(counts) > C:
        return _host_reference(hidden_states, ln1_w, qkv_w, o_w, ln2_w,
                               router_w, ws, w2s)
    moe_out = np.concatenate(
        [np.asarray(res.results[c]["moe_sl"], np.float32)
         for c in range(NCORES)], axis=0)                        # [T, H]
    residual = np.concatenate(
        [np.asarray(res.results[c]["resid_sl"], np.float32)
         for c in range(NCORES)], axis=0)                        # [T, H]
    return moe_out, residual
